# revision 44
# baseline (speedup 1.0000x reference)
"""Trainium2 Bass kernel for nn_AdaptiveDecoder (shared MLP + hard-routed type heads).

Strategy:
  * Host: sort nodes by type; split each type's count over 8 cores with minimal
    padding (per-type cap = ceil(count/8) rounded to 4) -> every core sees the
    SAME static layout of type-pure node-column blocks, so the compiled SPMD
    program bakes in the block->head mapping and the device does zero routing.
  * Device: activations stay transposed ([feature, nodes]) so the three matmul
    stages chain without transposes.  Non-GEMM PE work is ~3 cycles/col on top
    of the 112-cycle bf16 GEMM floor:
      - LayerNorm column sums via ONE fp8 DoubleRow ones-matmul (2 MAC
        planes/cycle): the hs/qs running sums are packed side-by-side in an
        fp8e4 tile, plane-0 of the two-plane ones-lhsT routes the hs sums to
        partitions {0,32,64} (ACT chain + rank-1 rhs homes) and plane-1
        routes the qs sum to partition 96.  fp8 quantization of the stats
        costs ~0.3% on sigma and ~4% on the (tiny) mean -- raises rel_err
        only 4.4e-3 -> 4.7e-3.  ACT partition reads must start at multiples
        of 32, and DoubleRow ldweights needs a 4B-aligned plane width, hence
        the [p, 2, 128] lhsT shape.
      - The -mu*c2 mean corrections (gamma folded into the head weights
        host-side) are two K=1 matmuls at PE rows 32/64 plus the 1/sigma
        broadcast at row 0 -- all three stream concurrently (disjoint PE row
        groups), so the tail is ~one slot.
  * All DRAM inputs are pre-tiled on the host into their exact SBUF layouts so
    every load is one dma_start with multi-KB contiguous rows (queue bandwidth
    collapses ~4x below 4KB rows, so startup pieces stay >= 512KB).
  * Startup DMA schedule: the first ~20us of compute needs ~4MB of weights
    (w1 1MB, w2 2MB, head 0.5MB) but the aggregate DMA fabric is ~358GB/s and
    HBM is contended by all 8 cores, so the two HWDGE queues carry everything
    early in strict first-use order (sync: xt0, w1-half2, w2 q1/q3;  scalar:
    w1-half1, w2 q0/q2, head type0) while the high-latency gpsimd SWDGE queue
    gets only late-needed assets.  b1/b2 memset instead of DMA'd when zero.
    Four warm matmuls sit between stage-1 halves of block 0 so the w1-half2
    arrival window ramps the clock instead of idling the PE.
  * LN chain: rsig = sqrt(1/(E[h^2]-mu^2+eps)) with ACT Square + ACT
    scale+eps-bias feeding one DVE sub + DVE reciprocal, and the final ACT
    Sqrt fusing the bf16 downcast.  negmu is ordered after the rsig-chain ACTs
    (it fills the DVE sub/recip window, and rsig is the end-of-kernel critical
    path).  hs/qs reductions run as an interleaved running sum inside the
    stage-2 m-loop, keeping the post-stage-2 serial depth at 2 DVE ops.
  * PSUM: the rsig-broadcast tile shares the 2-bank stats pool (stat tiles die
    before the deferred tail runs), freeing a 4th bank for the stage-1/2
    accumulators -- with 3 the PE hit ~50-300ns psum-rotation waits against
    the DVE relu drain.
  * GEMMs run bf16 (full PE rate; fp8 DoubleRow on the GEMMs measures
    rel_err ~7e-2 in simulation, over the 2e-2 gate; one fp8 stage alone is
    ~4e-2 -- fp8 is only admissible for the LN statistics).  First block is
    384 cols so its stage-1/2 pace the w2 stream-in; last block is 128 cols
    to shorten the final drain.
"""

import sys

sys.path.insert(0, "/opt/trn_rl_repo")

from contextlib import ExitStack

import numpy as np

N_CORES = 8
LATENT, HIDDEN, OUT, TYPES = 512, 1024, 256, 3
P = 128
NB = 512  # node columns per block (PSUM f32 bank limit)
KL = LATENT // P  # 4 k-tiles, stage 1
KH = HIDDEN // P  # 8 k-tiles, stage 2 / head
MH = HIDDEN // P  # 8 m-chunks of hidden
MO = OUT // P  # 2 m-chunks of head output
LN_EPS = 1e-5
MM_BF16 = True
N_WARM = 11  # PE clock-ramp dummy matmuls (HAM needs ~3.4us of activity)
N_WARM_MID = 4  # extra warms between block-0 stage-1 halves: they burn
# the w1-second-half DMA wait productively (clock ramp) instead of idling
B0 = 384  # first-block cols: big enough that stage-1/2 pace the w2 stream-in
# m-chunk processing order for stages 1/2 (natural: the m-major contiguous
# startup transfers arrive in this order anyway; DMA rows must stay >=4KB
# for full queue bandwidth, so chunks can't be fine-grained)
S1_ORDER = list(range(MH))
S2_ORDER = list(range(MH))


def _caps_from_counts(counts):
    caps = []
    for tt in range(TYPES):
        cap = -(-int(counts[tt]) // N_CORES)  # ceil
        cap = -(-cap // 4) * 4  # round to 4 cols (keeps DMA rows 8B-aligned)
        caps.append(cap)
    return caps


def _blocks_from_caps(caps):
    """Type-pure blocks tiling [0, R). Remainders split so blocks stay >=256;
    the overall first block is B0 so its stage-1/2 consume the streamed
    w2 no faster than the DMA queues supply it, and the overall last block is
    128 (shorter LN drain after the final matmul)."""
    blocks = []
    off = 0
    for tt in range(TYPES):
        cols = caps[tt]
        j = 0
        while j < cols:
            rem = cols - j
            if rem >= 2 * NB:
                nb = NB
            elif rem > NB:
                nb = -(-((rem + 1) // 2) // 4) * 4
            else:
                nb = rem
            blocks.append((tt, off + j, nb))
            j += nb
        off += cols
    if blocks and blocks[0][2] >= NB:
        t, c0, nb = blocks[0]
        blocks[0:1] = [(t, c0, B0), (t, c0 + B0, nb - B0)]
    if blocks and blocks[-1][2] >= 256:
        t, c0, nb = blocks[-1]
        blocks[-1:] = [(t, c0, nb - 128), (t, c0 + nb - 128, 128)]
    return blocks


def plan(node_types, pad_odd=True):
    """Host-side layout plan shared by all cores.

    Returns (blocks, R, caps, idx_by_type) where idx_by_type[t][c] is the array
    of original row indices of type t assigned to core c.
    """
    node_types = np.asarray(node_types)
    counts = np.bincount(node_types, minlength=TYPES)
    caps = _caps_from_counts(counts)
    idx_by_type = []
    order = np.argsort(node_types, kind="stable")
    starts = np.concatenate([[0], np.cumsum(counts)])
    for tt in range(TYPES):
        idx_t = order[starts[tt] : starts[tt + 1]]
        base, rem = divmod(int(counts[tt]), N_CORES)
        parts, o = [], 0
        for c in range(N_CORES):
            n = base + (1 if c < rem else 0)
            parts.append(idx_t[o : o + n])
            o += n
        idx_by_type.append(parts)
    R = sum(caps)
    blocks = _blocks_from_caps(caps)
    return blocks, R, caps, idx_by_type


def build_program(blocks, R, use_c1=True, use_b1=True, use_b2=True, mm_bf16=True):
    """blocks: list of (type_idx, col_offset, n_cols); R: node columns per core."""
    import concourse.mybir as mybir
    import concourse.tile as tile
    from concourse import bacc, bass_isa

    dt = mybir.dt
    f32, f32r, bf16 = dt.float32, dt.float32r, dt.bfloat16
    f8e4 = dt.float8e4
    mmdt = bf16 if mm_bf16 else f32r
    AF = mybir.ActivationFunctionType
    ALU = mybir.AluOpType

    nc = bacc.Bacc("TRN2", target_bir_lowering=False, debug=False, num_devices=N_CORES)

    xtd = nc.dram_tensor("xtp", [P, KL * R], mmdt, kind="ExternalInput").ap()
    w1d = nc.dram_tensor("w1p", [P, KL * HIDDEN], mmdt, kind="ExternalInput").ap()
    w2d = nc.dram_tensor("w2p", [P, KH * HIDDEN], mmdt, kind="ExternalInput").ap()
    whpd = nc.dram_tensor("whpp", [P, TYPES * KH * OUT], mmdt, kind="ExternalInput").ap()
    b1d = nc.dram_tensor("b1r", [P, MH], f32, kind="ExternalInput").ap()
    b2d = nc.dram_tensor("b2r", [P, MH], f32, kind="ExternalInput").ap()
    c1d = nc.dram_tensor("c1r", [1, TYPES * OUT], mmdt, kind="ExternalInput").ap()
    c2d = nc.dram_tensor("c2r", [1, TYPES * OUT], mmdt, kind="ExternalInput").ap()
    outd = nc.dram_tensor("out", [OUT, R], f32, kind="ExternalOutput").ap()

    def cv(ap):  # engine-facing view of an mm-dtype tile
        return ap if mm_bf16 else ap.bitcast(f32)

    with tile.TileContext(nc) as tc, ExitStack() as ctx:
        consts = ctx.enter_context(tc.tile_pool(name="consts", bufs=1))
        xt_pool = ctx.enter_context(tc.tile_pool(name="xt", bufs=3))
        h1_pool = ctx.enter_context(tc.tile_pool(name="h1", bufs=2))
        h2_pool = ctx.enter_context(tc.tile_pool(name="h2", bufs=2))
        sq_pool = ctx.enter_context(tc.tile_pool(name="sq", bufs=1))
        hs_pool = ctx.enter_context(tc.tile_pool(name="hs", bufs=2))
        qs_pool = ctx.enter_context(tc.tile_pool(name="qs", bufs=2))
        hq_pool = ctx.enter_context(tc.tile_pool(name="hq", bufs=2))
        rv_pool = ctx.enter_context(tc.tile_pool(name="rv", bufs=2))
        ab_pool = ctx.enter_context(tc.tile_pool(name="ab", bufs=2))
        out_pool = ctx.enter_context(tc.tile_pool(name="outp", bufs=2))
        ps_mlp = ctx.enter_context(tc.tile_pool(name="ps_mlp", bufs=4, space="PSUM"))
        ps_head = ctx.enter_context(tc.tile_pool(name="ps_head", bufs=2, space="PSUM"))
        # ps_stat also serves the rsig-broadcast tile: the stat tiles are
        # dead (negmu/musq/e2s read them at end-of-block) before the deferred
        # tail's broadcast matmul runs a block later, so 2 banks cover all 3
        # tiles and the freed bank gives ps_mlp a 4th buffer.
        ps_stat = ctx.enter_context(tc.tile_pool(name="ps_stat", bufs=2, space="PSUM"))

        # gpsimd carries late-needed startup weights; recurring xt loads
        # round-robin on the sync/scalar HWDGE queues
        dma_engines = [nc.sync, nc.scalar]
        dma_rr = [0]

        def dma(out, in_):
            eng = dma_engines[dma_rr[0] % len(dma_engines)]
            dma_rr[0] += 1
            eng.dma_start(out=out, in_=in_)

        def load_xt(c0, nb, eng=None):
            xt_t = xt_pool.tile([P, KL * NB], mmdt, tag="xt")
            if eng is None:
                dma(xt_t[:, : KL * nb], xtd[:, KL * c0 : KL * (c0 + nb)])
            else:
                eng.dma_start(out=xt_t[:, : KL * nb], in_=xtd[:, KL * c0 : KL * (c0 + nb)])
            return xt_t

        # --- PE warm-up: the HAM clock-gate needs ~3.4us of sustained PE
        # activity to release full clock; burn the DMA-wait window on dummy
        # matmuls over a memset scratch tile so the first real matmul runs at
        # 2.4 GHz instead of 1.2 ---
        warm_sb = consts.tile([P, NB], bf16)
        nc.vector.memset(warm_sb[:], 0.0)
        ps_w = ps_stat.tile([P, NB], f32, tag="stat")  # dummy psum, never consumed
        for _ in range(N_WARM):
            nc.tensor.matmul(
                ps_w[:], lhsT=warm_sb[:, :P], rhs=warm_sb[:], start=True, stop=True
            )

        # --- startup weight/input stream-in.  Three usable DMA queues (sync +
        # scalar HWDGE, gpsimd SWDGE with ~4.5us extra latency).  Queue
        # bandwidth depends on DMA row length: ~190GB/s at 4KB rows but only
        # ~50GB/s at 1KB rows, so transfers are 512KB pieces with 4KB rows
        # (w1/whp halves, w2 quarters), assigned per-queue in first-use order:
        #   sync:   xt0, w2q0, w2q2, xt1, [steady rr]
        #   scalar: w1h1, w1h2, whp(t0,mc0), whp(t0,mc1), xt2, [steady rr]
        #   gpsimd: [b1,b2 if nonzero], w2q1, w2q3, c2 rows, whp(t1/t2,*)
        w1_sb = consts.tile([P, KL * HIDDEN], mmdt)
        w2_sb = consts.tile([P, KH * HIDDEN], mmdt)
        whp_sb = consts.tile([P, TYPES * MO * KH * P], mmdt)
        t0_first = blocks[0][0] if blocks else 0
        type_order = [t0_first] + [t for t in range(TYPES) if t != t0_first]
        W1H = KL * HIDDEN // 2  # w1 half cols (512KB, 4KB rows)
        W2Q = KH * HIDDEN // 4  # w2 quarter cols (512KB, 4KB rows)

        def w2_quarter(eng, q):
            eng.dma_start(
                out=w2_sb[:, q * W2Q : (q + 1) * W2Q], in_=w2d[:, q * W2Q : (q + 1) * W2Q]
            )

        def whp_chunk(eng, t, mc):
            o = (t * MO + mc) * KH * P
            eng.dma_start(out=whp_sb[:, o : o + KH * P], in_=whpd[:, o : o + KH * P])

        b1_sb = consts.tile([P, MH], f32)
        b2_sb = consts.tile([P, MH], f32)
        if not use_b1:
            nc.vector.memset(b1_sb[:], 0.0)
        if not use_b2:
            nc.vector.memset(b2_sb[:], 0.0)

        xt_prefetch = {}
        # sync queue: xt0 then the second w1 half (its stage-1 m4 deadline is
        # tighter than any w2 quarter's)
        xt_prefetch[0] = load_xt(blocks[0][1], blocks[0][2], eng=nc.sync)
        nc.sync.dma_start(out=w1_sb[:, W1H:], in_=w1d[:, W1H:])
        for q in (1, 3):
            w2_quarter(nc.sync, q)
        if len(blocks) > 1:
            xt_prefetch[1] = load_xt(blocks[1][1], blocks[1][2], eng=nc.sync)
        # scalar queue
        nc.scalar.dma_start(out=w1_sb[:, :W1H], in_=w1d[:, :W1H])
        for q in (0, 2):
            w2_quarter(nc.scalar, q)
        for mc in range(MO):
            whp_chunk(nc.scalar, t0_first, mc)
        if len(blocks) > 2:
            xt_prefetch[2] = load_xt(blocks[2][1], blocks[2][2], eng=nc.scalar)
        # gpsimd queue (SWDGE): only tiny or late-needed assets -- its big
        # transfers would steal aggregate DMA-fabric bandwidth (~358GB/s/core)
        # from the critical early w1/w2 stream on the HWDGE queues
        if use_b1:
            nc.gpsimd.dma_start(out=b1_sb[:], in_=b1d[:])
        if use_b2:
            nc.gpsimd.dma_start(out=b2_sb[:], in_=b2d[:])
        # rank-1 constants live at partitions {32,64} = the PE rows that
        # consume them in the packed tail slot
        c2t = consts.tile([65, TYPES * OUT], mmdt)
        nc.gpsimd.dma_start(out=c2t[32:33, :], in_=c2d[:])
        nc.gpsimd.dma_start(out=c2t[64:65, :], in_=c2d[:])
        c1t = consts.tile([65, TYPES * OUT], mmdt)
        if use_c1:
            nc.gpsimd.dma_start(out=c1t[32:33, :], in_=c1d[:])
            nc.gpsimd.dma_start(out=c1t[64:65, :], in_=c1d[:])
        for t in type_order[1:]:
            for mc in range(MO):
                whp_chunk(nc.gpsimd, t, mc)

        onesr = consts.tile([1, P], mmdt)  # lhsT for the rsig broadcast (row group 0)
        nc.vector.memset(onesr[:], 1.0)
        # ones at lhsT cols {0,32,64}: the stats matmuls emit their column
        # sums at partitions 0/32/64 simultaneously (0: var chain; 32/64:
        # the rank-1 rhs row groups)
        ones65 = consts.tile([P, 65], bf16)
        nc.vector.memset(ones65[:], 0.0)
        for cc in (0, 32, 64):
            nc.vector.memset(ones65[:, cc : cc + 1], 1.0)
        # two-plane fp8 ones for the fused DoubleRow stats matmul: plane 0
        # (cols 0..64) routes the hs sums to partitions {0,32,64}, plane 1
        # (col 65+16) routes the qs sum to partition 16
        ones2 = consts.tile([P, 256], f8e4)
        nc.vector.memset(ones2[:], 0.0)
        for cc in (0, 32, 64):
            nc.vector.memset(ones2[:, cc : cc + 1], 1.0)
        nc.vector.memset(ones2[:, 224 : 225], 1.0)
        eps_c = consts.tile([P, 1], f32)
        nc.vector.memset(eps_c[:], LN_EPS)
        act_warm = consts.tile([1, 1], f32)
        nc.scalar.activation(act_warm[:], eps_c[0:1, :], AF.Sqrt)

        # --- per-block pipeline (software-pipelined: the LN-dependent DVE/ACT
        # tail of block b runs while block b+1's matmuls keep the PE hot) ---

        def emit_tail(t, c0, nb, ph_list, negmu_t, sv_t, rsig_t):
            # rank-1 corrections + rsig broadcast: three K=1 matmuls at PE
            # rows {0, 32, 64} (disjoint row groups -> they stream together).
            # rsig first: it is the end of the longest ACT/DVE chain, and once
            # it is ready negmu (earlier in the ACT FIFO) must also be ready.
            ps_a = ps_stat.tile([P, NB], f32, tag="stat")
            nc.tensor.matmul(
                ps_a[:, :nb], lhsT=onesr[:], rhs=rsig_t[0:1, :nb],
                start=True, stop=True,
            )
            nc.tensor.matmul(
                ph_list[0][:, :nb],
                lhsT=c2t[32:33, t * OUT : t * OUT + P],
                rhs=negmu_t[32:33, :nb],
                start=False,
                stop=not use_c1,
            )
            nc.tensor.matmul(
                ph_list[1][:, :nb],
                lhsT=c2t[64:65, t * OUT + P : t * OUT + 2 * P],
                rhs=negmu_t[64:65, :nb],
                start=False,
                stop=not use_c1,
            )
            if use_c1:
                nc.tensor.matmul(
                    ph_list[0][:, :nb],
                    lhsT=c1t[32:33, t * OUT : t * OUT + P],
                    rhs=sv_t[32:33, :nb],
                    start=False,
                    stop=True,
                )
                nc.tensor.matmul(
                    ph_list[1][:, :nb],
                    lhsT=c1t[64:65, t * OUT + P : t * OUT + 2 * P],
                    rhs=sv_t[64:65, :nb],
                    start=False,
                    stop=True,
                )
            a_sb = ab_pool.tile([P, NB], f32, tag="a")
            nc.scalar.activation(a_sb[:, :nb], ps_a[:, :nb], AF.Identity)
            out_sb = out_pool.tile([P, MO * NB], f32, tag="out")
            out_engs = (nc.sync, nc.scalar)
            for mc in range(MO):
                nc.vector.tensor_mul(
                    out_sb[:, mc * NB : mc * NB + nb], ph_list[mc][:, :nb],
                    a_sb[:, :nb],
                )
                out_engs[mc % 2].dma_start(
                    out=outd[mc * P : (mc + 1) * P, c0 : c0 + nb],
                    in_=out_sb[:, mc * NB : mc * NB + nb],
                )

        import functools

        pending = []
        for bi, (t, c0, nb) in enumerate(blocks):
            xt_t = xt_prefetch.pop(bi, None)
            if xt_t is None:
                xt_t = load_xt(c0, nb)

            # stage 1: h1^T = relu(W1^T x + b1)   [HIDDEN, nb]
            h1_t = h1_pool.tile([P, MH * NB], mmdt, tag="h1")
            for pos1, m in enumerate(S1_ORDER):
                if bi == 0 and pos1 == MH // 2:
                    for _ in range(N_WARM_MID):
                        nc.tensor.matmul(
                            ps_w[:], lhsT=warm_sb[:, :P], rhs=warm_sb[:],
                            start=True, stop=True,
                        )
                ps = ps_mlp.tile([P, NB], f32, tag="ps_mlp")
                for k in range(KL):
                    nc.tensor.matmul(
                        ps[:, :nb],
                        lhsT=w1_sb[:, m * (KL * P) + k * P : m * (KL * P) + (k + 1) * P],
                        rhs=xt_t[:, k * nb : (k + 1) * nb],
                        start=(k == 0),
                        stop=(k == KL - 1),
                    )
                nc.vector.tensor_scalar(
                    h1_t[:, m * NB : m * NB + nb],
                    ps[:, :nb],
                    b1_sb[:, m : m + 1],
                    0.0,
                    op0=mybir.AluOpType.add,
                    op1=mybir.AluOpType.max,
                )

            # the previous block's deferred LN tail slots in here: its PE
            # inputs (negmu/sv/rsig) became ready while this block's stage 1
            # ran, so the packed rank-1 slot never stalls the PE
            if pending:
                pending.pop(0)()

            # stage 2: h2^T = W2^T h1 + b2; squares and the hs/qs LN
            # reduction trees ride along per chunk so the stats matmuls can
            # fire right after the last m-chunk
            h2_t = h2_pool.tile([P, MH * NB], mmdt, tag="h2")
            sq_t = sq_pool.tile([P, MH * NB], bf16, tag="sq")
            hs_t = hs_pool.tile([P, (MH // 2) * NB], bf16, tag="hs")
            qs_t = qs_pool.tile([P, (MH // 2) * NB], bf16, tag="qs")
            hsq8 = hq_pool.tile([P, 2 * NB], f8e4, tag="hq")
            for pos, m in enumerate(S2_ORDER):
                ps = ps_mlp.tile([P, NB], f32, tag="ps_mlp")
                for k in range(KH):
                    nc.tensor.matmul(
                        ps[:, :nb],
                        lhsT=w2_sb[:, m * (KH * P) + k * P : m * (KH * P) + (k + 1) * P],
                        rhs=h1_t[:, k * NB : k * NB + nb],
                        start=(k == 0),
                        stop=(k == KH - 1),
                    )
                nc.scalar.activation(
                    h2_t[:, m * NB : m * NB + nb],
                    ps[:, :nb],
                    AF.Identity,
                    bias=b2_sb[:, m : m + 1],
                )
                nc.vector.tensor_mul(
                    sq_t[:, m * NB : m * NB + nb],
                    cv(h2_t[:, m * NB : m * NB + nb]),
                    cv(h2_t[:, m * NB : m * NB + nb]),
                )
                if pos % 2 == 1:
                    mp = S2_ORDER[pos - 1]  # pair with the previous chunk
                    # running sum in slot 0: pair -> slot (0 or 1), then
                    # accumulate slot1 into slot0.  Keeps the post-stage-2
                    # serial depth at 2 DVE ops (pair + accum) instead of a
                    # 3-level tree, which shortens the end-of-kernel drain.
                    j = 0 if pos == 1 else 1
                    nc.vector.tensor_add(
                        hs_t[:, j * NB : j * NB + nb],
                        cv(h2_t[:, mp * NB : mp * NB + nb]),
                        cv(h2_t[:, m * NB : m * NB + nb]),
                    )
                    nc.vector.tensor_add(
                        qs_t[:, j * NB : j * NB + nb],
                        sq_t[:, mp * NB : mp * NB + nb],
                        sq_t[:, m * NB : m * NB + nb],
                    )
                    if pos > 1:
                        # final accumulation lands in the packed fp8 tile
                        # (planes [hs | qs]) that feeds the fused DoubleRow
                        # stats matmul; fp8 quantization of the stats costs
                        # ~0.3% on sigma and ~4% on the (tiny) mean -- noise
                        last_l = (pos == MH - 1) and not use_c1
                        hdst = hsq8[:, :nb] if last_l else hs_t[:, :nb]
                        nc.vector.tensor_add(
                            hdst, hs_t[:, :nb], hs_t[:, NB : NB + nb]
                        )
                        qdst = hsq8[:, nb : 2 * nb] if last_l else qs_t[:, :nb]
                        nc.vector.tensor_add(
                            qdst, qs_t[:, :nb], qs_t[:, NB : NB + nb]
                        )

            def emit_head():
                # head matmuls: only need h2, so they keep the PE hot while
                # the stats chain runs on DVE/ACT
                ph_list = []
                for mc in range(MO):
                    ph = ps_head.tile([P, NB], f32, tag="head")
                    for k in range(KH):
                        nc.tensor.matmul(
                            ph[:, :nb],
                            lhsT=whp_sb[
                                :,
                                ((t * MO + mc) * KH + k) * P : (
                                    (t * MO + mc) * KH + k + 1
                                )
                                * P,
                            ],
                            rhs=h2_t[:, k * NB : k * NB + nb],
                            start=(k == 0),
                            stop=False,
                        )
                    ph_list.append(ph)
                return ph_list

            # last block: stats BEFORE the head so the rsig chain overlaps
            # the head matmuls (no following block hides the final chain)
            last_blk = bi == len(blocks) - 1
            if not last_blk:
                ph_list = emit_head()

            # LN stats: column sums of hs/qs via ones-matmuls; the Q7-free
            # partition placement {0,32,64} feeds the ACT chain and the
            # rank-1 tail rows directly
            if not use_c1:
                # both column-sum sets in ONE fp8 DoubleRow matmul (2 MAC
                # planes/cycle): hs sums land at partitions {0,32,64}, the qs
                # sum at partition 16 -- one PE slot instead of two
                # the last block's stats borrow an (idle by now) mlp psum
                # bank: the stat pool's banks are still draining the previous
                # tails at that point, which showed up as a ~1us WAW wait
                ps_s = (ps_mlp if last_blk else ps_stat).tile(
                    [P, NB], f32, tag="ps_mlp" if last_blk else "stat"
                )
                nc.tensor.matmul(
                    ps_s[:, :nb],
                    lhsT=ones2[:].rearrange("p (two m) -> p two m", two=2),
                    rhs=hsq8[:, : 2 * nb].rearrange("p (two c) -> p two c", two=2),
                    start=True, stop=True,
                    perf_mode=mybir.MatmulPerfMode.DoubleRow,
                )
                ps_q = ps_s  # qs sum lives at partition 16 of the same tile
            else:
                ps_s = ps_stat.tile([65, NB], f32, tag="stat")
                nc.tensor.matmul(
                    ps_s[:, :nb], lhsT=ones65[:], rhs=hs_t[:, :nb],
                    start=True, stop=True,
                )
                ps_q = ps_stat.tile([65, NB], f32, tag="stat")
                nc.tensor.matmul(
                    ps_q[:, :nb], lhsT=ones65[:], rhs=qs_t[:, :nb],
                    start=True, stop=True,
                )

            if last_blk:
                ph_list = emit_head()

            negmu_t = rv_pool.tile([65, NB], mmdt, tag="negmu")
            if use_c1:
                # general path computes negmu first (c2 pair is earliest in
                # the PE FIFO)
                nc.scalar.activation(
                    negmu_t[:, :nb], ps_s[:, :nb], AF.Identity,
                    scale=-1.0 / HIDDEN,
                )
            if use_c1:
                # general path: sv = sqrt(var+eps) on partitions 0..64
                musq_t = rv_pool.tile([65, NB], f32, tag="musq")
                nc.scalar.activation(
                    musq_t[:, :nb], ps_s[:, :nb], AF.Square, scale=1.0 / HIDDEN
                )
                varv_t = rv_pool.tile([65, NB], f32, tag="varv")
                nc.scalar.activation(
                    varv_t[:, :nb], ps_q[:, :nb], AF.Identity, scale=1.0 / HIDDEN
                )
                nc.vector.tensor_sub(
                    varv_t[:, :nb], varv_t[:, :nb], musq_t[:, :nb]
                )
                sv_t = rv_pool.tile([65, NB], mmdt, tag="sv")
                nc.scalar.activation(
                    sv_t[:, :nb], varv_t[:, :nb], AF.Sqrt, bias=eps_c[0:65, :]
                )
                svf_t = rv_pool.tile([1, NB], f32, tag="svf")
                nc.scalar.activation(
                    svf_t[:, :nb], varv_t[0:1, :nb], AF.Sqrt, bias=eps_c[0:1, :]
                )
                rsf_t = rv_pool.tile([1, NB], f32, tag="rsf")
                nc.vector.reciprocal_approx_fast(rsf_t[:, :nb], svf_t[:, :nb])
                rsig_t = rv_pool.tile([1, NB], mmdt, tag="rsig")
                nc.scalar.activation(rsig_t[:, :nb], rsf_t[:, :nb], AF.Identity)
            else:
                # fast path: rsig = sqrt(1/(E[h^2] - mu^2 + eps)).  ACT order
                # is musq, e2s, negmu, sqrt: the rsig chain (critical for the
                # final block) starts immediately, and negmu computes during
                # the DVE sub+recip window, so it delays nothing.
                sv_t = None
                musq_t = rv_pool.tile([1, NB], f32, tag="musq")
                nc.scalar.activation(
                    musq_t[:, :nb], ps_s[0:1, :nb], AF.Square, scale=1.0 / HIDDEN
                )
                e2s_t = rv_pool.tile([1, NB], f32, tag="e2s")
                nc.scalar.activation(  # E[h^2] + eps: the eps rides the bias
                    e2s_t[:, :nb], ps_q[96:97, :nb], AF.Identity,
                    scale=1.0 / HIDDEN, bias=eps_c[0:1, :],
                )
                veps_t = rv_pool.tile([1, NB], f32, tag="veps")
                nc.vector.tensor_sub(
                    veps_t[:, :nb], e2s_t[:, :nb], musq_t[:, :nb]
                )
                rv_t = rv_pool.tile([1, NB], f32, tag="rv")
                nc.vector.reciprocal_approx_fast(rv_t[:, :nb], veps_t[:, :nb])
                nc.scalar.activation(
                    negmu_t[:, :nb], ps_s[0:65, :nb], AF.Identity,
                    scale=-1.0 / HIDDEN,
                )
                rsig_t = rv_pool.tile([1, NB], mmdt, tag="rsig")
                nc.scalar.activation(rsig_t[:, :nb], rv_t[:, :nb], AF.Sqrt)

            pending.append(functools.partial(
                emit_tail, t, c0, nb, ph_list, negmu_t, sv_t, rsig_t
            ))

        for pf in pending:
            pf()

    nc.compile()
    return nc


def _tf32(x):
    """Round fp32 to TF32 (10-bit mantissa, round-to-nearest-even)."""
    u = np.ascontiguousarray(x, dtype=np.float32).view(np.uint32).copy()
    lsb = (u >> np.uint32(13)) & np.uint32(1)
    u += np.uint32(0x0FFF) + lsb
    u &= np.uint32(0xFFFFE000)
    return u.view(np.float32)


def _tile_cols(a, kt):
    """[kt*P, C] -> [P, kt*C] with col index = k*C + c (the SBUF layout)."""
    kp, C = a.shape
    assert kp == kt * P
    return np.ascontiguousarray(
        a.reshape(kt, P, C).transpose(1, 0, 2).reshape(P, kt * C)
    )


def _tile_cols_mmajor(a, kt):
    """[kt*P, mt*P] -> [P, mt*kt*P] with col index = m*(kt*P) + k*P + pp,
    so a contiguous column range covers a run of m-chunks for ALL k."""
    kp, C = a.shape
    assert kp == kt * P and C % P == 0
    mt = C // P
    return np.ascontiguousarray(
        a.reshape(kt, P, mt, P).transpose(1, 2, 0, 3).reshape(P, mt * kt * P)
    )


def prep_inputs(node_latent, w1, b1, w2, b2, ln_gamma, ln_beta, head_w, head_b,
                caps, idx_by_type, mm_bf16=True):
    """Build the 8 per-core input maps (everything pre-tiled to SBUF layout)."""
    if mm_bf16:
        import ml_dtypes

        cast = lambda a: np.asarray(a, dtype=np.float32).astype(ml_dtypes.bfloat16)
    else:
        cast = _tf32
    whp = np.asarray(ln_gamma)[:, None] * np.asarray(head_w)  # [T, H, OUT]
    # mc-major per type: col = ((t*MO + mc)*KH + k)*P + pp
    whpp = np.concatenate(
        [_tile_cols_mmajor(cast(whp[t]), KH) for t in range(TYPES)], axis=1
    )  # [P, T*MO*KH*P]
    c1 = cast(np.asarray(ln_beta @ head_w + head_b)).reshape(1, TYPES * OUT)
    c2 = cast(np.asarray(ln_gamma @ head_w)).reshape(1, TYPES * OUT)
    w1p = _tile_cols_mmajor(cast(w1), KL)  # [P, MH*KL*P], m-major
    w2p = _tile_cols_mmajor(cast(w2), KH)  # [P, MH*KH*P], m-major
    b1r = np.ascontiguousarray(np.asarray(b1).reshape(MH, P).T).astype(np.float32)
    b2r = np.ascontiguousarray(np.asarray(b2).reshape(MH, P).T).astype(np.float32)
    R = sum(caps)
    blocks = _blocks_from_caps(caps)
    node_latent = np.asarray(node_latent, dtype=np.float32)
    in_maps = []
    for c in range(N_CORES):
        xc = np.zeros((R, LATENT), np.float32)
        off = 0
        for tt in range(TYPES):
            idx = idx_by_type[tt][c]
            xc[off : off + len(idx)] = node_latent[idx]
            off += caps[tt]
        xcb = cast(xc)
        xtp = np.empty((P, KL * R), dtype=xcb.dtype)
        for (_t, c0, nb) in blocks:
            xtp[:, KL * c0 : KL * (c0 + nb)] = (
                xcb[c0 : c0 + nb, :].reshape(nb, KL, P).transpose(2, 1, 0)
                .reshape(P, KL * nb)
            )
        in_maps.append(
            {
                "xtp": xtp,
                "w1p": w1p,
                "w2p": w2p,
                "whpp": whpp,
                "b1r": b1r,
                "b2r": b2r,
                "c1r": c1,
                "c2r": c2,
            }
        )
    return in_maps


def unpack_outputs(results, caps, idx_by_type, n_rows):
    out = np.empty((n_rows, OUT), np.float32)
    for c in range(N_CORES):
        oc = results[c]["out"]  # [OUT, R]
        off = 0
        for tt in range(TYPES):
            idx = idx_by_type[tt][c]
            out[idx] = oc[:, off : off + len(idx)].T
            off += caps[tt]
    return out


def kernel(node_latent, node_types, w1, b1, w2, b2, ln_gamma, ln_beta, head_w, head_b):
    from concourse.bass_utils import run_bass_kernel_spmd

    node_latent = np.asarray(node_latent, dtype=np.float32)
    node_types = np.asarray(node_types)
    blocks, R, caps, idx_by_type = plan(node_types)
    use_c1 = bool(np.any(np.asarray(ln_beta @ head_w + head_b)))
    use_b1 = bool(np.any(np.asarray(b1)))
    use_b2 = bool(np.any(np.asarray(b2)))
    nc = build_program(
        blocks, R, use_c1=use_c1, use_b1=use_b1, use_b2=use_b2, mm_bf16=MM_BF16
    )
    in_maps = prep_inputs(
        node_latent, w1, b1, w2, b2, ln_gamma, ln_beta, head_w, head_b,
        caps, idx_by_type, mm_bf16=MM_BF16,
    )
    res = run_bass_kernel_spmd(nc, in_maps, core_ids=list(range(N_CORES)))
    return unpack_outputs(res.results, caps, idx_by_type, node_latent.shape[0])


# revision 45
# speedup vs baseline: 1.1964x; 1.1964x over previous
"""Trainium2 Bass kernel for nn_AdaptiveDecoder (shared MLP + hard-routed type heads).

Strategy:
  * Host: sort nodes by type; split each type's count over 8 cores with minimal
    padding (per-type cap = ceil(count/8) rounded to 4) -> every core sees the
    SAME static layout of type-pure node-column blocks, so the compiled SPMD
    program bakes in the block->head mapping and the device does zero routing.
  * Device: activations stay transposed ([feature, nodes]) so the three matmul
    stages chain without transposes.  Non-GEMM PE work is ~3 cycles/col on top
    of the 112-cycle bf16 GEMM floor:
      - LayerNorm column sums via ONE fp8 DoubleRow ones-matmul (2 MAC
        planes/cycle): the hs/qs running sums are packed side-by-side in an
        fp8e4 tile, plane-0 of the two-plane ones-lhsT routes the hs sums to
        partitions {0,32,64} (ACT chain + rank-1 rhs homes) and plane-1
        routes the qs sum to partition 96.  fp8 quantization of the stats
        costs ~0.3% on sigma and ~4% on the (tiny) mean -- raises rel_err
        only 4.4e-3 -> 4.7e-3.  ACT partition reads must start at multiples
        of 32, and DoubleRow ldweights needs a 4B-aligned plane width, hence
        the [p, 2, 128] lhsT shape.
      - The -mu*c2 mean corrections (gamma folded into the head weights
        host-side) are two K=1 matmuls at PE rows 32/64 plus the 1/sigma
        broadcast at row 0 -- all three stream concurrently (disjoint PE row
        groups), so the tail is ~one slot.
  * All DRAM inputs are pre-tiled on the host into their exact SBUF layouts so
    every load is one dma_start with multi-KB contiguous rows (queue bandwidth
    collapses ~4x below 4KB rows, so startup pieces stay >= 512KB).
  * Startup DMA schedule: the first ~20us of compute needs ~4MB of weights
    (w1 1MB, w2 2MB, head 0.5MB) but the aggregate DMA fabric is ~358GB/s and
    HBM is contended by all 8 cores, so the two HWDGE queues carry everything
    early in strict first-use order (sync: xt0, w1-half2, w2 q1/q3;  scalar:
    w1-half1, w2 q0/q2, head type0) while the high-latency gpsimd SWDGE queue
    gets only late-needed assets.  b1/b2 memset instead of DMA'd when zero.
    Four warm matmuls sit between stage-1 halves of block 0 so the w1-half2
    arrival window ramps the clock instead of idling the PE.
  * LN chain: rsig = sqrt(1/(E[h^2]-mu^2+eps)) with ACT Square + ACT
    scale+eps-bias feeding one DVE sub + DVE reciprocal, and the final ACT
    Sqrt fusing the bf16 downcast.  negmu is ordered after the rsig-chain ACTs
    (it fills the DVE sub/recip window, and rsig is the end-of-kernel critical
    path).  hs/qs reductions run as an interleaved running sum inside the
    stage-2 m-loop, keeping the post-stage-2 serial depth at 2 DVE ops.
  * PSUM: the rsig-broadcast tile shares the 2-bank stats pool (stat tiles die
    before the deferred tail runs), freeing a 4th bank for the stage-1/2
    accumulators -- with 3 the PE hit ~50-300ns psum-rotation waits against
    the DVE relu drain.
  * GEMMs run bf16 (full PE rate; fp8 DoubleRow on the GEMMs measures
    rel_err ~7e-2 in simulation, over the 2e-2 gate; one fp8 stage alone is
    ~4e-2 -- fp8 is only admissible for the LN statistics).  First block is
    384 cols so its stage-1/2 pace the w2 stream-in; last block is 128 cols
    to shorten the final drain.
"""

import sys

sys.path.insert(0, "/opt/trn_rl_repo")

from contextlib import ExitStack

import numpy as np

N_CORES = 8
LATENT, HIDDEN, OUT, TYPES = 512, 1024, 256, 3
P = 128
NB = 512  # node columns per block (PSUM f32 bank limit)
KL = LATENT // P  # 4 k-tiles, stage 1
KH = HIDDEN // P  # 8 k-tiles, stage 2 / head
MH = HIDDEN // P  # 8 m-chunks of hidden
MO = OUT // P  # 2 m-chunks of head output
LN_EPS = 1e-5
MM_BF16 = True
N_WARM = 11  # PE clock-ramp dummy matmuls (HAM needs ~3.4us of activity)
N_WARM_MID = 4  # extra warms between block-0 stage-1 halves: they burn
# the w1-second-half DMA wait productively (clock ramp) instead of idling
B0 = 384  # first-block cols: big enough that stage-1/2 pace the w2 stream-in
# m-chunk processing order for stages 1/2 (natural: the m-major contiguous
# startup transfers arrive in this order anyway; DMA rows must stay >=4KB
# for full queue bandwidth, so chunks can't be fine-grained)
S1_ORDER = list(range(MH))
S2_ORDER = list(range(MH))


def _caps_from_counts(counts):
    caps = []
    for tt in range(TYPES):
        cap = -(-int(counts[tt]) // N_CORES)  # ceil
        cap = -(-cap // 4) * 4  # round to 4 cols (keeps DMA rows 8B-aligned)
        caps.append(cap)
    return caps


def _blocks_from_caps(caps):
    """Type-pure blocks tiling [0, R). Remainders split so blocks stay >=256;
    the overall first block is B0 so its stage-1/2 consume the streamed
    w2 no faster than the DMA queues supply it, and the overall last block is
    128 (shorter LN drain after the final matmul)."""
    blocks = []
    off = 0
    for tt in range(TYPES):
        cols = caps[tt]
        j = 0
        while j < cols:
            rem = cols - j
            if rem >= 2 * NB:
                nb = NB
            elif rem > NB:
                nb = -(-((rem + 1) // 2) // 4) * 4
            else:
                nb = rem
            blocks.append((tt, off + j, nb))
            j += nb
        off += cols
    if blocks and blocks[0][2] >= NB:
        t, c0, nb = blocks[0]
        blocks[0:1] = [(t, c0, B0), (t, c0 + B0, nb - B0)]
    if blocks and blocks[-1][2] >= 256:
        t, c0, nb = blocks[-1]
        blocks[-1:] = [(t, c0, nb - 128), (t, c0 + nb - 128, 128)]
    return blocks


def plan(node_types, pad_odd=True):
    """Host-side layout plan shared by all cores.

    Returns (blocks, R, caps, idx_by_type) where idx_by_type[t][c] is the array
    of original row indices of type t assigned to core c.
    """
    node_types = np.asarray(node_types)
    counts = np.bincount(node_types, minlength=TYPES)
    caps = _caps_from_counts(counts)
    idx_by_type = []
    order = np.argsort(node_types, kind="stable")
    starts = np.concatenate([[0], np.cumsum(counts)])
    for tt in range(TYPES):
        idx_t = order[starts[tt] : starts[tt + 1]]
        base, rem = divmod(int(counts[tt]), N_CORES)
        parts, o = [], 0
        for c in range(N_CORES):
            n = base + (1 if c < rem else 0)
            parts.append(idx_t[o : o + n])
            o += n
        idx_by_type.append(parts)
    R = sum(caps)
    blocks = _blocks_from_caps(caps)
    return blocks, R, caps, idx_by_type


def build_program(blocks, R, use_c1=True, use_b1=True, use_b2=True, mm_bf16=True):
    """blocks: list of (type_idx, col_offset, n_cols); R: node columns per core."""
    import concourse.mybir as mybir
    import concourse.tile as tile
    from concourse import bacc, bass_isa

    dt = mybir.dt
    f32, f32r, bf16 = dt.float32, dt.float32r, dt.bfloat16
    f8e4 = dt.float8e4
    mmdt = bf16 if mm_bf16 else f32r
    AF = mybir.ActivationFunctionType
    ALU = mybir.AluOpType

    nc = bacc.Bacc("TRN2", target_bir_lowering=False, debug=False, num_devices=N_CORES)

    xtd = nc.dram_tensor("xtp", [P, KL * R], mmdt, kind="ExternalInput").ap()
    w1d = nc.dram_tensor("w1p", [P, KL * HIDDEN], mmdt, kind="ExternalInput").ap()
    w2d = nc.dram_tensor("w2p", [P, KH * HIDDEN], mmdt, kind="ExternalInput").ap()
    whpd = nc.dram_tensor("whpp", [P, TYPES * KH * OUT], mmdt, kind="ExternalInput").ap()
    b1d = nc.dram_tensor("b1r", [P, MH], f32, kind="ExternalInput").ap()
    b2d = nc.dram_tensor("b2r", [P, MH], f32, kind="ExternalInput").ap()
    c1d = nc.dram_tensor("c1r", [1, TYPES * OUT], mmdt, kind="ExternalInput").ap()
    c2d = nc.dram_tensor("c2r", [1, TYPES * OUT], mmdt, kind="ExternalInput").ap()
    outd = nc.dram_tensor("out", [OUT, R], f32, kind="ExternalOutput").ap()

    def cv(ap):  # engine-facing view of an mm-dtype tile
        return ap if mm_bf16 else ap.bitcast(f32)

    with tile.TileContext(nc) as tc, ExitStack() as ctx:
        consts = ctx.enter_context(tc.tile_pool(name="consts", bufs=1))
        xt_pool = ctx.enter_context(tc.tile_pool(name="xt", bufs=3))
        h1_pool = ctx.enter_context(tc.tile_pool(name="h1", bufs=2))
        h2_pool = ctx.enter_context(tc.tile_pool(name="h2", bufs=2))
        sq_pool = ctx.enter_context(tc.tile_pool(name="sq", bufs=1))
        hs_pool = ctx.enter_context(tc.tile_pool(name="hs", bufs=2))
        qs_pool = ctx.enter_context(tc.tile_pool(name="qs", bufs=2))
        hq_pool = ctx.enter_context(tc.tile_pool(name="hq", bufs=2))
        rv_pool = ctx.enter_context(tc.tile_pool(name="rv", bufs=2))
        ab_pool = ctx.enter_context(tc.tile_pool(name="ab", bufs=2))
        out_pool = ctx.enter_context(tc.tile_pool(name="outp", bufs=2))
        ps_mlp = ctx.enter_context(tc.tile_pool(name="ps_mlp", bufs=4, space="PSUM"))
        ps_head = ctx.enter_context(tc.tile_pool(name="ps_head", bufs=2, space="PSUM"))
        # ps_stat also serves the rsig-broadcast tile: the stat tiles are
        # dead (negmu/musq/e2s read them at end-of-block) before the deferred
        # tail's broadcast matmul runs a block later, so 2 banks cover all 3
        # tiles and the freed bank gives ps_mlp a 4th buffer.
        ps_stat = ctx.enter_context(tc.tile_pool(name="ps_stat", bufs=2, space="PSUM"))

        # gpsimd carries late-needed startup weights; recurring xt loads
        # round-robin on the sync/scalar HWDGE queues
        dma_engines = [nc.sync, nc.scalar]
        dma_rr = [0]

        def dma(out, in_):
            eng = dma_engines[dma_rr[0] % len(dma_engines)]
            dma_rr[0] += 1
            eng.dma_start(out=out, in_=in_)

        def load_xt(c0, nb, eng=None):
            xt_t = xt_pool.tile([P, KL * NB], mmdt, tag="xt")
            if eng is None:
                dma(xt_t[:, : KL * nb], xtd[:, KL * c0 : KL * (c0 + nb)])
            else:
                eng.dma_start(out=xt_t[:, : KL * nb], in_=xtd[:, KL * c0 : KL * (c0 + nb)])
            return xt_t

        # --- PE warm-up: the HAM clock-gate needs ~3.4us of sustained PE
        # activity to release full clock; burn the DMA-wait window on dummy
        # matmuls over a memset scratch tile so the first real matmul runs at
        # 2.4 GHz instead of 1.2 ---
        warm_sb = consts.tile([P, NB], bf16)
        nc.vector.memset(warm_sb[:], 0.0)
        ps_w = ps_stat.tile([P, NB], f32, tag="stat")  # dummy psum, never consumed
        for _ in range(N_WARM):
            nc.tensor.matmul(
                ps_w[:], lhsT=warm_sb[:, :P], rhs=warm_sb[:], start=True, stop=True
            )

        # --- startup weight/input stream-in.  Three usable DMA queues (sync +
        # scalar HWDGE, gpsimd SWDGE with ~4.5us extra latency).  Queue
        # bandwidth depends on DMA row length: ~190GB/s at 4KB rows but only
        # ~50GB/s at 1KB rows, so transfers are 512KB pieces with 4KB rows
        # (w1/whp halves, w2 quarters), assigned per-queue in first-use order:
        #   sync:   xt0, w2q0, w2q2, xt1, [steady rr]
        #   scalar: w1h1, w1h2, whp(t0,mc0), whp(t0,mc1), xt2, [steady rr]
        #   gpsimd: [b1,b2 if nonzero], w2q1, w2q3, c2 rows, whp(t1/t2,*)
        w1_sb = consts.tile([P, KL * HIDDEN], mmdt)
        w2_sb = consts.tile([P, KH * HIDDEN], mmdt)
        whp_sb = consts.tile([P, TYPES * MO * KH * P], mmdt)
        t0_first = blocks[0][0] if blocks else 0
        type_order = [t0_first] + [t for t in range(TYPES) if t != t0_first]
        W1H = KL * HIDDEN // 2  # w1 half cols (512KB, 4KB rows)
        W2Q = KH * HIDDEN // 4  # w2 quarter cols (512KB, 4KB rows)

        def w2_quarter(eng, q):
            eng.dma_start(
                out=w2_sb[:, q * W2Q : (q + 1) * W2Q], in_=w2d[:, q * W2Q : (q + 1) * W2Q]
            )

        def whp_chunk(eng, t, mc):
            o = (t * MO + mc) * KH * P
            eng.dma_start(out=whp_sb[:, o : o + KH * P], in_=whpd[:, o : o + KH * P])

        b1_sb = consts.tile([P, MH], f32)
        b2_sb = consts.tile([P, MH], f32)
        if not use_b1:
            nc.vector.memset(b1_sb[:], 0.0)
        if not use_b2:
            nc.vector.memset(b2_sb[:], 0.0)

        xt_prefetch = {}
        # sync queue: xt0 then the second w1 half (its stage-1 m4 deadline is
        # tighter than any w2 quarter's)
        xt_prefetch[0] = load_xt(blocks[0][1], blocks[0][2], eng=nc.sync)
        nc.sync.dma_start(out=w1_sb[:, W1H:], in_=w1d[:, W1H:])
        for q in (1, 3):
            w2_quarter(nc.sync, q)
        if len(blocks) > 1:
            xt_prefetch[1] = load_xt(blocks[1][1], blocks[1][2], eng=nc.sync)
        # scalar queue
        nc.scalar.dma_start(out=w1_sb[:, :W1H], in_=w1d[:, :W1H])
        for q in (0, 2):
            w2_quarter(nc.scalar, q)
        for mc in range(MO):
            whp_chunk(nc.scalar, t0_first, mc)
        if len(blocks) > 2:
            xt_prefetch[2] = load_xt(blocks[2][1], blocks[2][2], eng=nc.scalar)
        # gpsimd queue (SWDGE): only tiny or late-needed assets -- its big
        # transfers would steal aggregate DMA-fabric bandwidth (~358GB/s/core)
        # from the critical early w1/w2 stream on the HWDGE queues
        if use_b1:
            nc.gpsimd.dma_start(out=b1_sb[:], in_=b1d[:])
        if use_b2:
            nc.gpsimd.dma_start(out=b2_sb[:], in_=b2d[:])
        # rank-1 constants live at partitions {32,64} = the PE rows that
        # consume them in the packed tail slot
        c2t = consts.tile([65, TYPES * OUT], mmdt)
        nc.gpsimd.dma_start(out=c2t[32:33, :], in_=c2d[:])
        nc.gpsimd.dma_start(out=c2t[64:65, :], in_=c2d[:])
        c1t = consts.tile([65, TYPES * OUT], mmdt)
        if use_c1:
            nc.gpsimd.dma_start(out=c1t[32:33, :], in_=c1d[:])
            nc.gpsimd.dma_start(out=c1t[64:65, :], in_=c1d[:])
        for t in type_order[1:]:
            for mc in range(MO):
                whp_chunk(nc.gpsimd, t, mc)

        onesr = consts.tile([1, P], mmdt)  # lhsT for the rsig broadcast (row group 0)
        nc.vector.memset(onesr[:], 1.0)
        # ones at lhsT cols {0,32,64}: the stats matmuls emit their column
        # sums at partitions 0/32/64 simultaneously (0: var chain; 32/64:
        # the rank-1 rhs row groups)
        ones65 = consts.tile([P, 65], bf16)
        nc.vector.memset(ones65[:], 0.0)
        for cc in (0, 32, 64):
            nc.vector.memset(ones65[:, cc : cc + 1], 1.0)
        # two-plane fp8 ones for the fused DoubleRow stats matmul: plane 0
        # (cols 0..64) routes the hs sums to partitions {0,32,64}, plane 1
        # (col 65+16) routes the qs sum to partition 16
        ones2 = consts.tile([P, 256], f8e4)
        nc.vector.memset(ones2[:], 0.0)
        for cc in (0, 32, 64):
            nc.vector.memset(ones2[:, cc : cc + 1], 1.0)
        nc.vector.memset(ones2[:, 224 : 225], 1.0)
        eps_c = consts.tile([P, 1], f32)
        nc.vector.memset(eps_c[:], LN_EPS)
        act_warm = consts.tile([1, 1], f32)
        nc.scalar.activation(act_warm[:], eps_c[0:1, :], AF.Sqrt)

        # --- per-block pipeline (software-pipelined: the LN-dependent DVE/ACT
        # tail of block b runs while block b+1's matmuls keep the PE hot) ---

        def emit_tail(t, c0, nb, ph_list, negmu_t, sv_t, rsig_t):
            # rank-1 corrections + rsig broadcast: three K=1 matmuls at PE
            # rows {0, 32, 64} (disjoint row groups -> they stream together).
            # rsig first: it is the end of the longest ACT/DVE chain, and once
            # it is ready negmu (earlier in the ACT FIFO) must also be ready.
            ps_a = ps_stat.tile([P, NB], f32, tag="stat")
            nc.tensor.matmul(
                ps_a[:, :nb], lhsT=onesr[:], rhs=rsig_t[0:1, :nb],
                start=True, stop=True,
            )
            nc.tensor.matmul(
                ph_list[0][:, :nb],
                lhsT=c2t[32:33, t * OUT : t * OUT + P],
                rhs=negmu_t[32:33, :nb],
                start=False,
                stop=not use_c1,
            )
            nc.tensor.matmul(
                ph_list[1][:, :nb],
                lhsT=c2t[64:65, t * OUT + P : t * OUT + 2 * P],
                rhs=negmu_t[64:65, :nb],
                start=False,
                stop=not use_c1,
            )
            if use_c1:
                nc.tensor.matmul(
                    ph_list[0][:, :nb],
                    lhsT=c1t[32:33, t * OUT : t * OUT + P],
                    rhs=sv_t[32:33, :nb],
                    start=False,
                    stop=True,
                )
                nc.tensor.matmul(
                    ph_list[1][:, :nb],
                    lhsT=c1t[64:65, t * OUT + P : t * OUT + 2 * P],
                    rhs=sv_t[64:65, :nb],
                    start=False,
                    stop=True,
                )
            a_sb = ab_pool.tile([P, NB], f32, tag="a")
            nc.scalar.activation(a_sb[:, :nb], ps_a[:, :nb], AF.Identity)
            out_sb = out_pool.tile([P, MO * NB], f32, tag="out")
            out_engs = (nc.sync, nc.scalar)
            for mc in range(MO):
                nc.vector.tensor_mul(
                    out_sb[:, mc * NB : mc * NB + nb], ph_list[mc][:, :nb],
                    a_sb[:, :nb],
                )
                out_engs[mc % 2].dma_start(
                    out=outd[mc * P : (mc + 1) * P, c0 : c0 + nb],
                    in_=out_sb[:, mc * NB : mc * NB + nb],
                )

        import functools

        pending = []
        for bi, (t, c0, nb) in enumerate(blocks):
            xt_t = xt_prefetch.pop(bi, None)
            if xt_t is None:
                xt_t = load_xt(c0, nb)

            # stage 1: h1^T = relu(W1^T x + b1)   [HIDDEN, nb]
            h1_t = h1_pool.tile([P, MH * NB], mmdt, tag="h1")
            for pos1, m in enumerate(S1_ORDER):
                if bi == 0 and pos1 == MH // 2:
                    for _ in range(N_WARM_MID):
                        nc.tensor.matmul(
                            ps_w[:], lhsT=warm_sb[:, :P], rhs=warm_sb[:],
                            start=True, stop=True,
                        )
                ps = ps_mlp.tile([P, NB], f32, tag="ps_mlp")
                for k in range(KL):
                    nc.tensor.matmul(
                        ps[:, :nb],
                        lhsT=w1_sb[:, m * (KL * P) + k * P : m * (KL * P) + (k + 1) * P],
                        rhs=xt_t[:, k * nb : (k + 1) * nb],
                        start=(k == 0),
                        stop=(k == KL - 1),
                    )
                nc.vector.tensor_scalar(
                    h1_t[:, m * NB : m * NB + nb],
                    ps[:, :nb],
                    b1_sb[:, m : m + 1],
                    0.0,
                    op0=mybir.AluOpType.add,
                    op1=mybir.AluOpType.max,
                )

            # the previous block's deferred LN tail slots in here: its PE
            # inputs (negmu/sv/rsig) became ready while this block's stage 1
            # ran, so the packed rank-1 slot never stalls the PE
            if pending:
                pending.pop(0)()

            # stage 2: h2^T = W2^T h1 + b2; squares and the hs/qs LN
            # reduction trees ride along per chunk so the stats matmuls can
            # fire right after the last m-chunk
            h2_t = h2_pool.tile([P, MH * NB], mmdt, tag="h2")
            sq_t = sq_pool.tile([P, MH * NB], bf16, tag="sq")
            hs_t = hs_pool.tile([P, (MH // 2) * NB], bf16, tag="hs")
            qs_t = qs_pool.tile([P, (MH // 2) * NB], bf16, tag="qs")
            hsq8 = hq_pool.tile([P, 2 * NB], f8e4, tag="hq")
            for pos, m in enumerate(S2_ORDER):
                ps = ps_mlp.tile([P, NB], f32, tag="ps_mlp")
                for k in range(KH):
                    nc.tensor.matmul(
                        ps[:, :nb],
                        lhsT=w2_sb[:, m * (KH * P) + k * P : m * (KH * P) + (k + 1) * P],
                        rhs=h1_t[:, k * NB : k * NB + nb],
                        start=(k == 0),
                        stop=(k == KH - 1),
                    )
                nc.scalar.activation(
                    h2_t[:, m * NB : m * NB + nb],
                    ps[:, :nb],
                    AF.Identity,
                    bias=b2_sb[:, m : m + 1],
                )
                nc.vector.tensor_mul(
                    sq_t[:, m * NB : m * NB + nb],
                    cv(h2_t[:, m * NB : m * NB + nb]),
                    cv(h2_t[:, m * NB : m * NB + nb]),
                )
                if pos % 2 == 1:
                    mp = S2_ORDER[pos - 1]  # pair with the previous chunk
                    # running sum in slot 0: pair -> slot (0 or 1), then
                    # accumulate slot1 into slot0.  Keeps the post-stage-2
                    # serial depth at 2 DVE ops (pair + accum) instead of a
                    # 3-level tree, which shortens the end-of-kernel drain.
                    j = 0 if pos == 1 else 1
                    nc.vector.tensor_add(
                        hs_t[:, j * NB : j * NB + nb],
                        cv(h2_t[:, mp * NB : mp * NB + nb]),
                        cv(h2_t[:, m * NB : m * NB + nb]),
                    )
                    nc.vector.tensor_add(
                        qs_t[:, j * NB : j * NB + nb],
                        sq_t[:, mp * NB : mp * NB + nb],
                        sq_t[:, m * NB : m * NB + nb],
                    )
                    if pos > 1:
                        # final accumulation lands in the packed fp8 tile
                        # (planes [hs | qs]) that feeds the fused DoubleRow
                        # stats matmul; fp8 quantization of the stats costs
                        # ~0.3% on sigma and ~4% on the (tiny) mean -- noise
                        last_l = (pos == MH - 1) and not use_c1
                        hdst = hsq8[:, :nb] if last_l else hs_t[:, :nb]
                        nc.vector.tensor_add(
                            hdst, hs_t[:, :nb], hs_t[:, NB : NB + nb]
                        )
                        qdst = hsq8[:, nb : 2 * nb] if last_l else qs_t[:, :nb]
                        nc.vector.tensor_add(
                            qdst, qs_t[:, :nb], qs_t[:, NB : NB + nb]
                        )

            def emit_head():
                # head matmuls: only need h2, so they keep the PE hot while
                # the stats chain runs on DVE/ACT
                ph_list = []
                for mc in range(MO):
                    ph = ps_head.tile([P, NB], f32, tag="head")
                    for k in range(KH):
                        nc.tensor.matmul(
                            ph[:, :nb],
                            lhsT=whp_sb[
                                :,
                                ((t * MO + mc) * KH + k) * P : (
                                    (t * MO + mc) * KH + k + 1
                                )
                                * P,
                            ],
                            rhs=h2_t[:, k * NB : k * NB + nb],
                            start=(k == 0),
                            stop=False,
                        )
                    ph_list.append(ph)
                return ph_list

            # last block: stats BEFORE the head so the rsig chain overlaps
            # the head matmuls (no following block hides the final chain)
            last_blk = bi == len(blocks) - 1
            if not last_blk:
                ph_list = emit_head()

            # LN stats: column sums of hs/qs via ones-matmuls; the Q7-free
            # partition placement {0,32,64} feeds the ACT chain and the
            # rank-1 tail rows directly
            if not use_c1:
                # both column-sum sets in ONE fp8 DoubleRow matmul (2 MAC
                # planes/cycle): hs sums land at partitions {0,32,64}, the qs
                # sum at partition 16 -- one PE slot instead of two
                ps_s = ps_stat.tile([P, NB], f32, tag="stat")
                nc.tensor.matmul(
                    ps_s[:, :nb],
                    lhsT=ones2[:].rearrange("p (two m) -> p two m", two=2),
                    rhs=hsq8[:, : 2 * nb].rearrange("p (two c) -> p two c", two=2),
                    start=True, stop=True,
                    perf_mode=mybir.MatmulPerfMode.DoubleRow,
                )
                ps_q = ps_s  # qs sum lives at partition 16 of the same tile
            else:
                ps_s = ps_stat.tile([65, NB], f32, tag="stat")
                nc.tensor.matmul(
                    ps_s[:, :nb], lhsT=ones65[:], rhs=hs_t[:, :nb],
                    start=True, stop=True,
                )
                ps_q = ps_stat.tile([65, NB], f32, tag="stat")
                nc.tensor.matmul(
                    ps_q[:, :nb], lhsT=ones65[:], rhs=qs_t[:, :nb],
                    start=True, stop=True,
                )

            if last_blk:
                ph_list = emit_head()

            negmu_t = rv_pool.tile([65, NB], mmdt, tag="negmu")
            if use_c1:
                # general path computes negmu first (c2 pair is earliest in
                # the PE FIFO)
                nc.scalar.activation(
                    negmu_t[:, :nb], ps_s[:, :nb], AF.Identity,
                    scale=-1.0 / HIDDEN,
                )
            if use_c1:
                # general path: sv = sqrt(var+eps) on partitions 0..64
                musq_t = rv_pool.tile([65, NB], f32, tag="musq")
                nc.scalar.activation(
                    musq_t[:, :nb], ps_s[:, :nb], AF.Square, scale=1.0 / HIDDEN
                )
                varv_t = rv_pool.tile([65, NB], f32, tag="varv")
                nc.scalar.activation(
                    varv_t[:, :nb], ps_q[:, :nb], AF.Identity, scale=1.0 / HIDDEN
                )
                nc.vector.tensor_sub(
                    varv_t[:, :nb], varv_t[:, :nb], musq_t[:, :nb]
                )
                sv_t = rv_pool.tile([65, NB], mmdt, tag="sv")
                nc.scalar.activation(
                    sv_t[:, :nb], varv_t[:, :nb], AF.Sqrt, bias=eps_c[0:65, :]
                )
                svf_t = rv_pool.tile([1, NB], f32, tag="svf")
                nc.scalar.activation(
                    svf_t[:, :nb], varv_t[0:1, :nb], AF.Sqrt, bias=eps_c[0:1, :]
                )
                rsf_t = rv_pool.tile([1, NB], f32, tag="rsf")
                nc.vector.reciprocal_approx_fast(rsf_t[:, :nb], svf_t[:, :nb])
                rsig_t = rv_pool.tile([1, NB], mmdt, tag="rsig")
                nc.scalar.activation(rsig_t[:, :nb], rsf_t[:, :nb], AF.Identity)
            else:
                # fast path: rsig = sqrt(1/(E[h^2] - mu^2 + eps)).  ACT order
                # is musq, e2s, negmu, sqrt: the rsig chain (critical for the
                # final block) starts immediately, and negmu computes during
                # the DVE sub+recip window, so it delays nothing.
                sv_t = None
                musq_t = rv_pool.tile([1, NB], f32, tag="musq")
                nc.scalar.activation(
                    musq_t[:, :nb], ps_s[0:1, :nb], AF.Square, scale=1.0 / HIDDEN
                )
                e2s_t = rv_pool.tile([1, NB], f32, tag="e2s")
                nc.scalar.activation(  # E[h^2] + eps: the eps rides the bias
                    e2s_t[:, :nb], ps_q[96:97, :nb], AF.Identity,
                    scale=1.0 / HIDDEN, bias=eps_c[0:1, :],
                )
                veps_t = rv_pool.tile([1, NB], f32, tag="veps")
                nc.vector.tensor_sub(
                    veps_t[:, :nb], e2s_t[:, :nb], musq_t[:, :nb]
                )
                rv_t = rv_pool.tile([1, NB], f32, tag="rv")
                nc.vector.reciprocal_approx_fast(rv_t[:, :nb], veps_t[:, :nb])
                nc.scalar.activation(
                    negmu_t[:, :nb], ps_s[0:65, :nb], AF.Identity,
                    scale=-1.0 / HIDDEN,
                )
                rsig_t = rv_pool.tile([1, NB], mmdt, tag="rsig")
                nc.scalar.activation(rsig_t[:, :nb], rv_t[:, :nb], AF.Sqrt)

            pending.append(functools.partial(
                emit_tail, t, c0, nb, ph_list, negmu_t, sv_t, rsig_t
            ))

        for pf in pending:
            pf()

    nc.compile()
    return nc


def _tf32(x):
    """Round fp32 to TF32 (10-bit mantissa, round-to-nearest-even)."""
    u = np.ascontiguousarray(x, dtype=np.float32).view(np.uint32).copy()
    lsb = (u >> np.uint32(13)) & np.uint32(1)
    u += np.uint32(0x0FFF) + lsb
    u &= np.uint32(0xFFFFE000)
    return u.view(np.float32)


def _tile_cols(a, kt):
    """[kt*P, C] -> [P, kt*C] with col index = k*C + c (the SBUF layout)."""
    kp, C = a.shape
    assert kp == kt * P
    return np.ascontiguousarray(
        a.reshape(kt, P, C).transpose(1, 0, 2).reshape(P, kt * C)
    )


def _tile_cols_mmajor(a, kt):
    """[kt*P, mt*P] -> [P, mt*kt*P] with col index = m*(kt*P) + k*P + pp,
    so a contiguous column range covers a run of m-chunks for ALL k."""
    kp, C = a.shape
    assert kp == kt * P and C % P == 0
    mt = C // P
    return np.ascontiguousarray(
        a.reshape(kt, P, mt, P).transpose(1, 2, 0, 3).reshape(P, mt * kt * P)
    )


def prep_inputs(node_latent, w1, b1, w2, b2, ln_gamma, ln_beta, head_w, head_b,
                caps, idx_by_type, mm_bf16=True):
    """Build the 8 per-core input maps (everything pre-tiled to SBUF layout)."""
    if mm_bf16:
        import ml_dtypes

        cast = lambda a: np.asarray(a, dtype=np.float32).astype(ml_dtypes.bfloat16)
    else:
        cast = _tf32
    whp = np.asarray(ln_gamma)[:, None] * np.asarray(head_w)  # [T, H, OUT]
    # mc-major per type: col = ((t*MO + mc)*KH + k)*P + pp
    whpp = np.concatenate(
        [_tile_cols_mmajor(cast(whp[t]), KH) for t in range(TYPES)], axis=1
    )  # [P, T*MO*KH*P]
    c1 = cast(np.asarray(ln_beta @ head_w + head_b)).reshape(1, TYPES * OUT)
    c2 = cast(np.asarray(ln_gamma @ head_w)).reshape(1, TYPES * OUT)
    w1p = _tile_cols_mmajor(cast(w1), KL)  # [P, MH*KL*P], m-major
    w2p = _tile_cols_mmajor(cast(w2), KH)  # [P, MH*KH*P], m-major
    b1r = np.ascontiguousarray(np.asarray(b1).reshape(MH, P).T).astype(np.float32)
    b2r = np.ascontiguousarray(np.asarray(b2).reshape(MH, P).T).astype(np.float32)
    R = sum(caps)
    blocks = _blocks_from_caps(caps)
    node_latent = np.asarray(node_latent, dtype=np.float32)
    in_maps = []
    for c in range(N_CORES):
        xc = np.zeros((R, LATENT), np.float32)
        off = 0
        for tt in range(TYPES):
            idx = idx_by_type[tt][c]
            xc[off : off + len(idx)] = node_latent[idx]
            off += caps[tt]
        xcb = cast(xc)
        xtp = np.empty((P, KL * R), dtype=xcb.dtype)
        for (_t, c0, nb) in blocks:
            xtp[:, KL * c0 : KL * (c0 + nb)] = (
                xcb[c0 : c0 + nb, :].reshape(nb, KL, P).transpose(2, 1, 0)
                .reshape(P, KL * nb)
            )
        in_maps.append(
            {
                "xtp": xtp,
                "w1p": w1p,
                "w2p": w2p,
                "whpp": whpp,
                "b1r": b1r,
                "b2r": b2r,
                "c1r": c1,
                "c2r": c2,
            }
        )
    return in_maps


def unpack_outputs(results, caps, idx_by_type, n_rows):
    out = np.empty((n_rows, OUT), np.float32)
    for c in range(N_CORES):
        oc = results[c]["out"]  # [OUT, R]
        off = 0
        for tt in range(TYPES):
            idx = idx_by_type[tt][c]
            out[idx] = oc[:, off : off + len(idx)].T
            off += caps[tt]
    return out


def kernel(node_latent, node_types, w1, b1, w2, b2, ln_gamma, ln_beta, head_w, head_b):
    from concourse.bass_utils import run_bass_kernel_spmd

    node_latent = np.asarray(node_latent, dtype=np.float32)
    node_types = np.asarray(node_types)
    blocks, R, caps, idx_by_type = plan(node_types)
    use_c1 = bool(np.any(np.asarray(ln_beta @ head_w + head_b)))
    use_b1 = bool(np.any(np.asarray(b1)))
    use_b2 = bool(np.any(np.asarray(b2)))
    nc = build_program(
        blocks, R, use_c1=use_c1, use_b1=use_b1, use_b2=use_b2, mm_bf16=MM_BF16
    )
    in_maps = prep_inputs(
        node_latent, w1, b1, w2, b2, ln_gamma, ln_beta, head_w, head_b,
        caps, idx_by_type, mm_bf16=MM_BF16,
    )
    res = run_bass_kernel_spmd(nc, in_maps, core_ids=list(range(N_CORES)))
    return unpack_outputs(res.results, caps, idx_by_type, node_latent.shape[0])


# revision 46
# speedup vs baseline: 1.1973x; 1.0008x over previous
"""Trainium2 Bass kernel for nn_AdaptiveDecoder (shared MLP + hard-routed type heads).

Strategy:
  * Host: sort nodes by type; split each type's count over 8 cores with minimal
    padding (per-type cap = ceil(count/8) rounded to 4) -> every core sees the
    SAME static layout of type-pure node-column blocks, so the compiled SPMD
    program bakes in the block->head mapping and the device does zero routing.
  * Device: activations stay transposed ([feature, nodes]) so the three matmul
    stages chain without transposes.  Non-GEMM PE work is ~3 cycles/col on top
    of the 112-cycle bf16 GEMM floor:
      - LayerNorm column sums via ONE fp8 DoubleRow ones-matmul (2 MAC
        planes/cycle): the hs/qs running sums are packed side-by-side in an
        fp8e4 tile, plane-0 of the two-plane ones-lhsT routes the hs sums to
        partitions {0,32,64} (ACT chain + rank-1 rhs homes) and plane-1
        routes the qs sum to partition 96.  fp8 quantization of the stats
        costs ~0.3% on sigma and ~4% on the (tiny) mean -- raises rel_err
        only 4.4e-3 -> 4.7e-3.  ACT partition reads must start at multiples
        of 32, and DoubleRow ldweights needs a 4B-aligned plane width, hence
        the [p, 2, 128] lhsT shape.
      - The -mu*c2 mean corrections (gamma folded into the head weights
        host-side) are two K=1 matmuls at PE rows 32/64 plus the 1/sigma
        broadcast at row 0 -- all three stream concurrently (disjoint PE row
        groups), so the tail is ~one slot.
  * All DRAM inputs are pre-tiled on the host into their exact SBUF layouts so
    every load is one dma_start with multi-KB contiguous rows (queue bandwidth
    collapses ~4x below 4KB rows, so startup pieces stay >= 512KB).
  * Startup DMA schedule: the first ~20us of compute needs ~4MB of weights
    (w1 1MB, w2 2MB, head 0.5MB) but the aggregate DMA fabric is ~358GB/s and
    HBM is contended by all 8 cores, so the two HWDGE queues carry everything
    early in strict first-use order (sync: xt0, w1-half2, w2 q1/q3;  scalar:
    w1-half1, w2 q0/q2, head type0) while the high-latency gpsimd SWDGE queue
    gets only late-needed assets.  b1/b2 memset instead of DMA'd when zero.
    Four warm matmuls sit between stage-1 halves of block 0 so the w1-half2
    arrival window ramps the clock instead of idling the PE.
  * LN chain: rsig = sqrt(1/(E[h^2]-mu^2+eps)) with ACT Square + ACT
    scale+eps-bias feeding one DVE sub + DVE reciprocal, and the final ACT
    Sqrt fusing the bf16 downcast.  negmu is ordered after the rsig-chain ACTs
    (it fills the DVE sub/recip window, and rsig is the end-of-kernel critical
    path).  hs/qs reductions run as an interleaved running sum inside the
    stage-2 m-loop, keeping the post-stage-2 serial depth at 2 DVE ops.
  * PSUM: the rsig-broadcast tile shares the 2-bank stats pool (stat tiles die
    before the deferred tail runs), freeing a 4th bank for the stage-1/2
    accumulators -- with 3 the PE hit ~50-300ns psum-rotation waits against
    the DVE relu drain.
  * GEMMs run bf16 (full PE rate; fp8 DoubleRow on the GEMMs measures
    rel_err ~7e-2 in simulation, over the 2e-2 gate; one fp8 stage alone is
    ~4e-2 -- fp8 is only admissible for the LN statistics).  First block is
    384 cols so its stage-1/2 pace the w2 stream-in; last block is 128 cols
    to shorten the final drain.
"""

import sys

sys.path.insert(0, "/opt/trn_rl_repo")

from contextlib import ExitStack

import numpy as np

N_CORES = 8
LATENT, HIDDEN, OUT, TYPES = 512, 1024, 256, 3
P = 128
NB = 512  # node columns per block (PSUM f32 bank limit)
KL = LATENT // P  # 4 k-tiles, stage 1
KH = HIDDEN // P  # 8 k-tiles, stage 2 / head
MH = HIDDEN // P  # 8 m-chunks of hidden
MO = OUT // P  # 2 m-chunks of head output
LN_EPS = 1e-5
MM_BF16 = True
N_WARM = 12  # PE clock-ramp dummy matmuls (HAM needs ~3.4us of activity)
N_WARM_MID = 4  # extra warms between block-0 stage-1 halves: they burn
# the w1-second-half DMA wait productively (clock ramp) instead of idling
B0 = 384  # first-block cols: big enough that stage-1/2 pace the w2 stream-in
# m-chunk processing order for stages 1/2 (natural: the m-major contiguous
# startup transfers arrive in this order anyway; DMA rows must stay >=4KB
# for full queue bandwidth, so chunks can't be fine-grained)
S1_ORDER = list(range(MH))
S2_ORDER = list(range(MH))


def _caps_from_counts(counts):
    caps = []
    for tt in range(TYPES):
        cap = -(-int(counts[tt]) // N_CORES)  # ceil
        cap = -(-cap // 4) * 4  # round to 4 cols (keeps DMA rows 8B-aligned)
        caps.append(cap)
    return caps


def _blocks_from_caps(caps):
    """Type-pure blocks tiling [0, R). Remainders split so blocks stay >=256;
    the overall first block is B0 so its stage-1/2 consume the streamed
    w2 no faster than the DMA queues supply it, and the overall last block is
    128 (shorter LN drain after the final matmul)."""
    blocks = []
    off = 0
    for tt in range(TYPES):
        cols = caps[tt]
        j = 0
        while j < cols:
            rem = cols - j
            if rem >= 2 * NB:
                nb = NB
            elif rem > NB:
                nb = -(-((rem + 1) // 2) // 4) * 4
            else:
                nb = rem
            blocks.append((tt, off + j, nb))
            j += nb
        off += cols
    if blocks and blocks[0][2] >= NB:
        t, c0, nb = blocks[0]
        blocks[0:1] = [(t, c0, B0), (t, c0 + B0, nb - B0)]
    if blocks and blocks[-1][2] >= 256:
        t, c0, nb = blocks[-1]
        blocks[-1:] = [(t, c0, nb - 128), (t, c0 + nb - 128, 128)]
    return blocks


def plan(node_types, pad_odd=True):
    """Host-side layout plan shared by all cores.

    Returns (blocks, R, caps, idx_by_type) where idx_by_type[t][c] is the array
    of original row indices of type t assigned to core c.
    """
    node_types = np.asarray(node_types)
    counts = np.bincount(node_types, minlength=TYPES)
    caps = _caps_from_counts(counts)
    idx_by_type = []
    order = np.argsort(node_types, kind="stable")
    starts = np.concatenate([[0], np.cumsum(counts)])
    for tt in range(TYPES):
        idx_t = order[starts[tt] : starts[tt + 1]]
        base, rem = divmod(int(counts[tt]), N_CORES)
        parts, o = [], 0
        for c in range(N_CORES):
            n = base + (1 if c < rem else 0)
            parts.append(idx_t[o : o + n])
            o += n
        idx_by_type.append(parts)
    R = sum(caps)
    blocks = _blocks_from_caps(caps)
    return blocks, R, caps, idx_by_type


def build_program(blocks, R, use_c1=True, use_b1=True, use_b2=True, mm_bf16=True):
    """blocks: list of (type_idx, col_offset, n_cols); R: node columns per core."""
    import concourse.mybir as mybir
    import concourse.tile as tile
    from concourse import bacc, bass_isa

    dt = mybir.dt
    f32, f32r, bf16 = dt.float32, dt.float32r, dt.bfloat16
    f8e4 = dt.float8e4
    mmdt = bf16 if mm_bf16 else f32r
    AF = mybir.ActivationFunctionType
    ALU = mybir.AluOpType

    nc = bacc.Bacc("TRN2", target_bir_lowering=False, debug=False, num_devices=N_CORES)

    xtd = nc.dram_tensor("xtp", [P, KL * R], mmdt, kind="ExternalInput").ap()
    w1d = nc.dram_tensor("w1p", [P, KL * HIDDEN], mmdt, kind="ExternalInput").ap()
    w2d = nc.dram_tensor("w2p", [P, KH * HIDDEN], mmdt, kind="ExternalInput").ap()
    whpd = nc.dram_tensor("whpp", [P, TYPES * KH * OUT], mmdt, kind="ExternalInput").ap()
    b1d = nc.dram_tensor("b1r", [P, MH], f32, kind="ExternalInput").ap()
    b2d = nc.dram_tensor("b2r", [P, MH], f32, kind="ExternalInput").ap()
    c1d = nc.dram_tensor("c1r", [1, TYPES * OUT], mmdt, kind="ExternalInput").ap()
    c2d = nc.dram_tensor("c2r", [1, TYPES * OUT], mmdt, kind="ExternalInput").ap()
    outd = nc.dram_tensor("out", [OUT, R], f32, kind="ExternalOutput").ap()

    def cv(ap):  # engine-facing view of an mm-dtype tile
        return ap if mm_bf16 else ap.bitcast(f32)

    with tile.TileContext(nc) as tc, ExitStack() as ctx:
        consts = ctx.enter_context(tc.tile_pool(name="consts", bufs=1))
        xt_pool = ctx.enter_context(tc.tile_pool(name="xt", bufs=3))
        h1_pool = ctx.enter_context(tc.tile_pool(name="h1", bufs=2))
        h2_pool = ctx.enter_context(tc.tile_pool(name="h2", bufs=2))
        sq_pool = ctx.enter_context(tc.tile_pool(name="sq", bufs=1))
        hs_pool = ctx.enter_context(tc.tile_pool(name="hs", bufs=2))
        qs_pool = ctx.enter_context(tc.tile_pool(name="qs", bufs=2))
        hq_pool = ctx.enter_context(tc.tile_pool(name="hq", bufs=2))
        rv_pool = ctx.enter_context(tc.tile_pool(name="rv", bufs=2))
        ab_pool = ctx.enter_context(tc.tile_pool(name="ab", bufs=2))
        out_pool = ctx.enter_context(tc.tile_pool(name="outp", bufs=2))
        ps_mlp = ctx.enter_context(tc.tile_pool(name="ps_mlp", bufs=4, space="PSUM"))
        ps_head = ctx.enter_context(tc.tile_pool(name="ps_head", bufs=2, space="PSUM"))
        # ps_stat also serves the rsig-broadcast tile: the stat tiles are
        # dead (negmu/musq/e2s read them at end-of-block) before the deferred
        # tail's broadcast matmul runs a block later, so 2 banks cover all 3
        # tiles and the freed bank gives ps_mlp a 4th buffer.
        ps_stat = ctx.enter_context(tc.tile_pool(name="ps_stat", bufs=2, space="PSUM"))

        # gpsimd carries late-needed startup weights; recurring xt loads
        # round-robin on the sync/scalar HWDGE queues
        dma_engines = [nc.sync, nc.scalar]
        dma_rr = [0]

        def dma(out, in_):
            eng = dma_engines[dma_rr[0] % len(dma_engines)]
            dma_rr[0] += 1
            eng.dma_start(out=out, in_=in_)

        def load_xt(c0, nb, eng=None):
            xt_t = xt_pool.tile([P, KL * NB], mmdt, tag="xt")
            if eng is None:
                dma(xt_t[:, : KL * nb], xtd[:, KL * c0 : KL * (c0 + nb)])
            else:
                eng.dma_start(out=xt_t[:, : KL * nb], in_=xtd[:, KL * c0 : KL * (c0 + nb)])
            return xt_t

        # --- PE warm-up: the HAM clock-gate needs ~3.4us of sustained PE
        # activity to release full clock; burn the DMA-wait window on dummy
        # matmuls over a memset scratch tile so the first real matmul runs at
        # 2.4 GHz instead of 1.2 ---
        warm_sb = consts.tile([P, NB], bf16)
        nc.vector.memset(warm_sb[:], 0.0)
        ps_w = ps_stat.tile([P, NB], f32, tag="stat")  # dummy psum, never consumed
        for _ in range(N_WARM):
            nc.tensor.matmul(
                ps_w[:], lhsT=warm_sb[:, :P], rhs=warm_sb[:], start=True, stop=True
            )

        # --- startup weight/input stream-in.  Three usable DMA queues (sync +
        # scalar HWDGE, gpsimd SWDGE with ~4.5us extra latency).  Queue
        # bandwidth depends on DMA row length: ~190GB/s at 4KB rows but only
        # ~50GB/s at 1KB rows, so transfers are 512KB pieces with 4KB rows
        # (w1/whp halves, w2 quarters), assigned per-queue in first-use order:
        #   sync:   xt0, w2q0, w2q2, xt1, [steady rr]
        #   scalar: w1h1, w1h2, whp(t0,mc0), whp(t0,mc1), xt2, [steady rr]
        #   gpsimd: [b1,b2 if nonzero], w2q1, w2q3, c2 rows, whp(t1/t2,*)
        w1_sb = consts.tile([P, KL * HIDDEN], mmdt)
        w2_sb = consts.tile([P, KH * HIDDEN], mmdt)
        whp_sb = consts.tile([P, TYPES * MO * KH * P], mmdt)
        t0_first = blocks[0][0] if blocks else 0
        type_order = [t0_first] + [t for t in range(TYPES) if t != t0_first]
        W1H = KL * HIDDEN // 2  # w1 half cols (512KB, 4KB rows)
        W2Q = KH * HIDDEN // 4  # w2 quarter cols (512KB, 4KB rows)

        def w2_quarter(eng, q):
            eng.dma_start(
                out=w2_sb[:, q * W2Q : (q + 1) * W2Q], in_=w2d[:, q * W2Q : (q + 1) * W2Q]
            )

        def whp_chunk(eng, t, mc):
            o = (t * MO + mc) * KH * P
            eng.dma_start(out=whp_sb[:, o : o + KH * P], in_=whpd[:, o : o + KH * P])

        b1_sb = consts.tile([P, MH], f32)
        b2_sb = consts.tile([P, MH], f32)
        if not use_b1:
            nc.vector.memset(b1_sb[:], 0.0)
        if not use_b2:
            nc.vector.memset(b2_sb[:], 0.0)

        xt_prefetch = {}
        # sync queue: xt0 then the second w1 half (its stage-1 m4 deadline is
        # tighter than any w2 quarter's)
        xt_prefetch[0] = load_xt(blocks[0][1], blocks[0][2], eng=nc.sync)
        nc.sync.dma_start(out=w1_sb[:, W1H:], in_=w1d[:, W1H:])
        for q in (1, 3):
            w2_quarter(nc.sync, q)
        if len(blocks) > 1:
            xt_prefetch[1] = load_xt(blocks[1][1], blocks[1][2], eng=nc.sync)
        # scalar queue
        nc.scalar.dma_start(out=w1_sb[:, :W1H], in_=w1d[:, :W1H])
        for q in (0, 2):
            w2_quarter(nc.scalar, q)
        for mc in range(MO):
            whp_chunk(nc.scalar, t0_first, mc)
        if len(blocks) > 2:
            xt_prefetch[2] = load_xt(blocks[2][1], blocks[2][2], eng=nc.scalar)
        # gpsimd queue (SWDGE): only tiny or late-needed assets -- its big
        # transfers would steal aggregate DMA-fabric bandwidth (~358GB/s/core)
        # from the critical early w1/w2 stream on the HWDGE queues
        if use_b1:
            nc.gpsimd.dma_start(out=b1_sb[:], in_=b1d[:])
        if use_b2:
            nc.gpsimd.dma_start(out=b2_sb[:], in_=b2d[:])
        # rank-1 constants live at partitions {32,64} = the PE rows that
        # consume them in the packed tail slot
        c2t = consts.tile([65, TYPES * OUT], mmdt)
        nc.gpsimd.dma_start(out=c2t[32:33, :], in_=c2d[:])
        nc.gpsimd.dma_start(out=c2t[64:65, :], in_=c2d[:])
        c1t = consts.tile([65, TYPES * OUT], mmdt)
        if use_c1:
            nc.gpsimd.dma_start(out=c1t[32:33, :], in_=c1d[:])
            nc.gpsimd.dma_start(out=c1t[64:65, :], in_=c1d[:])
        for t in type_order[1:]:
            for mc in range(MO):
                whp_chunk(nc.gpsimd, t, mc)

        onesr = consts.tile([1, P], mmdt)  # lhsT for the rsig broadcast (row group 0)
        nc.vector.memset(onesr[:], 1.0)
        # ones at lhsT cols {0,32,64}: the stats matmuls emit their column
        # sums at partitions 0/32/64 simultaneously (0: var chain; 32/64:
        # the rank-1 rhs row groups)
        ones65 = consts.tile([P, 65], bf16)
        nc.vector.memset(ones65[:], 0.0)
        for cc in (0, 32, 64):
            nc.vector.memset(ones65[:, cc : cc + 1], 1.0)
        # two-plane fp8 ones for the fused DoubleRow stats matmul: plane 0
        # (cols 0..64) routes the hs sums to partitions {0,32,64}, plane 1
        # (col 65+16) routes the qs sum to partition 16
        ones2 = consts.tile([P, 256], f8e4)
        nc.vector.memset(ones2[:], 0.0)
        for cc in (0, 32, 64):
            nc.vector.memset(ones2[:, cc : cc + 1], 1.0)
        nc.vector.memset(ones2[:, 224 : 225], 1.0)
        eps_c = consts.tile([P, 1], f32)
        nc.vector.memset(eps_c[:], LN_EPS)
        act_warm = consts.tile([1, 1], f32)
        nc.scalar.activation(act_warm[:], eps_c[0:1, :], AF.Sqrt)

        # --- per-block pipeline (software-pipelined: the LN-dependent DVE/ACT
        # tail of block b runs while block b+1's matmuls keep the PE hot) ---

        def emit_tail(t, c0, nb, ph_list, negmu_t, sv_t, rsig_t):
            # rank-1 corrections + rsig broadcast: three K=1 matmuls at PE
            # rows {0, 32, 64} (disjoint row groups -> they stream together).
            # rsig first: it is the end of the longest ACT/DVE chain, and once
            # it is ready negmu (earlier in the ACT FIFO) must also be ready.
            ps_a = ps_stat.tile([P, NB], f32, tag="stat")
            nc.tensor.matmul(
                ps_a[:, :nb], lhsT=onesr[:], rhs=rsig_t[0:1, :nb],
                start=True, stop=True,
            )
            nc.tensor.matmul(
                ph_list[0][:, :nb],
                lhsT=c2t[32:33, t * OUT : t * OUT + P],
                rhs=negmu_t[32:33, :nb],
                start=False,
                stop=not use_c1,
            )
            nc.tensor.matmul(
                ph_list[1][:, :nb],
                lhsT=c2t[64:65, t * OUT + P : t * OUT + 2 * P],
                rhs=negmu_t[64:65, :nb],
                start=False,
                stop=not use_c1,
            )
            if use_c1:
                nc.tensor.matmul(
                    ph_list[0][:, :nb],
                    lhsT=c1t[32:33, t * OUT : t * OUT + P],
                    rhs=sv_t[32:33, :nb],
                    start=False,
                    stop=True,
                )
                nc.tensor.matmul(
                    ph_list[1][:, :nb],
                    lhsT=c1t[64:65, t * OUT + P : t * OUT + 2 * P],
                    rhs=sv_t[64:65, :nb],
                    start=False,
                    stop=True,
                )
            a_sb = ab_pool.tile([P, NB], f32, tag="a")
            nc.scalar.activation(a_sb[:, :nb], ps_a[:, :nb], AF.Identity)
            out_sb = out_pool.tile([P, MO * NB], f32, tag="out")
            out_engs = (nc.sync, nc.scalar)
            for mc in range(MO):
                nc.vector.tensor_mul(
                    out_sb[:, mc * NB : mc * NB + nb], ph_list[mc][:, :nb],
                    a_sb[:, :nb],
                )
                out_engs[mc % 2].dma_start(
                    out=outd[mc * P : (mc + 1) * P, c0 : c0 + nb],
                    in_=out_sb[:, mc * NB : mc * NB + nb],
                )

        import functools

        pending = []
        for bi, (t, c0, nb) in enumerate(blocks):
            xt_t = xt_prefetch.pop(bi, None)
            if xt_t is None:
                xt_t = load_xt(c0, nb)

            # stage 1: h1^T = relu(W1^T x + b1)   [HIDDEN, nb]
            h1_t = h1_pool.tile([P, MH * NB], mmdt, tag="h1")
            for pos1, m in enumerate(S1_ORDER):
                if bi == 0 and pos1 == MH // 2:
                    for _ in range(N_WARM_MID):
                        nc.tensor.matmul(
                            ps_w[:], lhsT=warm_sb[:, :P], rhs=warm_sb[:],
                            start=True, stop=True,
                        )
                ps = ps_mlp.tile([P, NB], f32, tag="ps_mlp")
                for k in range(KL):
                    nc.tensor.matmul(
                        ps[:, :nb],
                        lhsT=w1_sb[:, m * (KL * P) + k * P : m * (KL * P) + (k + 1) * P],
                        rhs=xt_t[:, k * nb : (k + 1) * nb],
                        start=(k == 0),
                        stop=(k == KL - 1),
                    )
                nc.vector.tensor_scalar(
                    h1_t[:, m * NB : m * NB + nb],
                    ps[:, :nb],
                    b1_sb[:, m : m + 1],
                    0.0,
                    op0=mybir.AluOpType.add,
                    op1=mybir.AluOpType.max,
                )

            # the previous block's deferred LN tail slots in here: its PE
            # inputs (negmu/sv/rsig) became ready while this block's stage 1
            # ran, so the packed rank-1 slot never stalls the PE
            if pending:
                pending.pop(0)()

            # stage 2: h2^T = W2^T h1 + b2; squares and the hs/qs LN
            # reduction trees ride along per chunk so the stats matmuls can
            # fire right after the last m-chunk
            h2_t = h2_pool.tile([P, MH * NB], mmdt, tag="h2")
            sq_t = sq_pool.tile([P, MH * NB], bf16, tag="sq")
            hs_t = hs_pool.tile([P, (MH // 2) * NB], bf16, tag="hs")
            qs_t = qs_pool.tile([P, (MH // 2) * NB], bf16, tag="qs")
            hsq8 = hq_pool.tile([P, 2 * NB], f8e4, tag="hq")
            for pos, m in enumerate(S2_ORDER):
                ps = ps_mlp.tile([P, NB], f32, tag="ps_mlp")
                for k in range(KH):
                    nc.tensor.matmul(
                        ps[:, :nb],
                        lhsT=w2_sb[:, m * (KH * P) + k * P : m * (KH * P) + (k + 1) * P],
                        rhs=h1_t[:, k * NB : k * NB + nb],
                        start=(k == 0),
                        stop=(k == KH - 1),
                    )
                nc.scalar.activation(
                    h2_t[:, m * NB : m * NB + nb],
                    ps[:, :nb],
                    AF.Identity,
                    bias=b2_sb[:, m : m + 1],
                )
                nc.vector.tensor_mul(
                    sq_t[:, m * NB : m * NB + nb],
                    cv(h2_t[:, m * NB : m * NB + nb]),
                    cv(h2_t[:, m * NB : m * NB + nb]),
                )
                if pos % 2 == 1:
                    mp = S2_ORDER[pos - 1]  # pair with the previous chunk
                    # running sum in slot 0: pair -> slot (0 or 1), then
                    # accumulate slot1 into slot0.  Keeps the post-stage-2
                    # serial depth at 2 DVE ops (pair + accum) instead of a
                    # 3-level tree, which shortens the end-of-kernel drain.
                    j = 0 if pos == 1 else 1
                    nc.vector.tensor_add(
                        hs_t[:, j * NB : j * NB + nb],
                        cv(h2_t[:, mp * NB : mp * NB + nb]),
                        cv(h2_t[:, m * NB : m * NB + nb]),
                    )
                    nc.vector.tensor_add(
                        qs_t[:, j * NB : j * NB + nb],
                        sq_t[:, mp * NB : mp * NB + nb],
                        sq_t[:, m * NB : m * NB + nb],
                    )
                    if pos > 1:
                        # final accumulation lands in the packed fp8 tile
                        # (planes [hs | qs]) that feeds the fused DoubleRow
                        # stats matmul; fp8 quantization of the stats costs
                        # ~0.3% on sigma and ~4% on the (tiny) mean -- noise
                        last_l = (pos == MH - 1) and not use_c1
                        hdst = hsq8[:, :nb] if last_l else hs_t[:, :nb]
                        nc.vector.tensor_add(
                            hdst, hs_t[:, :nb], hs_t[:, NB : NB + nb]
                        )
                        qdst = hsq8[:, nb : 2 * nb] if last_l else qs_t[:, :nb]
                        nc.vector.tensor_add(
                            qdst, qs_t[:, :nb], qs_t[:, NB : NB + nb]
                        )

            def emit_head():
                # head matmuls: only need h2, so they keep the PE hot while
                # the stats chain runs on DVE/ACT
                ph_list = []
                for mc in range(MO):
                    ph = ps_head.tile([P, NB], f32, tag="head")
                    for k in range(KH):
                        nc.tensor.matmul(
                            ph[:, :nb],
                            lhsT=whp_sb[
                                :,
                                ((t * MO + mc) * KH + k) * P : (
                                    (t * MO + mc) * KH + k + 1
                                )
                                * P,
                            ],
                            rhs=h2_t[:, k * NB : k * NB + nb],
                            start=(k == 0),
                            stop=False,
                        )
                    ph_list.append(ph)
                return ph_list

            # last block: stats BEFORE the head so the rsig chain overlaps
            # the head matmuls (no following block hides the final chain)
            last_blk = bi == len(blocks) - 1
            if not last_blk:
                ph_list = emit_head()

            # LN stats: column sums of hs/qs via ones-matmuls; the Q7-free
            # partition placement {0,32,64} feeds the ACT chain and the
            # rank-1 tail rows directly
            if not use_c1:
                # both column-sum sets in ONE fp8 DoubleRow matmul (2 MAC
                # planes/cycle): hs sums land at partitions {0,32,64}, the qs
                # sum at partition 16 -- one PE slot instead of two
                ps_s = ps_stat.tile([P, NB], f32, tag="stat")
                nc.tensor.matmul(
                    ps_s[:, :nb],
                    lhsT=ones2[:].rearrange("p (two m) -> p two m", two=2),
                    rhs=hsq8[:, : 2 * nb].rearrange("p (two c) -> p two c", two=2),
                    start=True, stop=True,
                    perf_mode=mybir.MatmulPerfMode.DoubleRow,
                )
                ps_q = ps_s  # qs sum lives at partition 16 of the same tile
            else:
                ps_s = ps_stat.tile([65, NB], f32, tag="stat")
                nc.tensor.matmul(
                    ps_s[:, :nb], lhsT=ones65[:], rhs=hs_t[:, :nb],
                    start=True, stop=True,
                )
                ps_q = ps_stat.tile([65, NB], f32, tag="stat")
                nc.tensor.matmul(
                    ps_q[:, :nb], lhsT=ones65[:], rhs=qs_t[:, :nb],
                    start=True, stop=True,
                )

            if last_blk:
                ph_list = emit_head()

            negmu_t = rv_pool.tile([65, NB], mmdt, tag="negmu")
            if use_c1:
                # general path computes negmu first (c2 pair is earliest in
                # the PE FIFO)
                nc.scalar.activation(
                    negmu_t[:, :nb], ps_s[:, :nb], AF.Identity,
                    scale=-1.0 / HIDDEN,
                )
            if use_c1:
                # general path: sv = sqrt(var+eps) on partitions 0..64
                musq_t = rv_pool.tile([65, NB], f32, tag="musq")
                nc.scalar.activation(
                    musq_t[:, :nb], ps_s[:, :nb], AF.Square, scale=1.0 / HIDDEN
                )
                varv_t = rv_pool.tile([65, NB], f32, tag="varv")
                nc.scalar.activation(
                    varv_t[:, :nb], ps_q[:, :nb], AF.Identity, scale=1.0 / HIDDEN
                )
                nc.vector.tensor_sub(
                    varv_t[:, :nb], varv_t[:, :nb], musq_t[:, :nb]
                )
                sv_t = rv_pool.tile([65, NB], mmdt, tag="sv")
                nc.scalar.activation(
                    sv_t[:, :nb], varv_t[:, :nb], AF.Sqrt, bias=eps_c[0:65, :]
                )
                svf_t = rv_pool.tile([1, NB], f32, tag="svf")
                nc.scalar.activation(
                    svf_t[:, :nb], varv_t[0:1, :nb], AF.Sqrt, bias=eps_c[0:1, :]
                )
                rsf_t = rv_pool.tile([1, NB], f32, tag="rsf")
                nc.vector.reciprocal_approx_fast(rsf_t[:, :nb], svf_t[:, :nb])
                rsig_t = rv_pool.tile([1, NB], mmdt, tag="rsig")
                nc.scalar.activation(rsig_t[:, :nb], rsf_t[:, :nb], AF.Identity)
            else:
                # fast path: rsig = sqrt(1/(E[h^2] - mu^2 + eps)).  ACT order
                # is musq, e2s, negmu, sqrt: the rsig chain (critical for the
                # final block) starts immediately, and negmu computes during
                # the DVE sub+recip window, so it delays nothing.
                sv_t = None
                musq_t = rv_pool.tile([1, NB], f32, tag="musq")
                nc.scalar.activation(
                    musq_t[:, :nb], ps_s[0:1, :nb], AF.Square, scale=1.0 / HIDDEN
                )
                e2s_t = rv_pool.tile([1, NB], f32, tag="e2s")
                nc.scalar.activation(  # E[h^2] + eps: the eps rides the bias
                    e2s_t[:, :nb], ps_q[96:97, :nb], AF.Identity,
                    scale=1.0 / HIDDEN, bias=eps_c[0:1, :],
                )
                veps_t = rv_pool.tile([1, NB], f32, tag="veps")
                nc.vector.tensor_sub(
                    veps_t[:, :nb], e2s_t[:, :nb], musq_t[:, :nb]
                )
                rv_t = rv_pool.tile([1, NB], f32, tag="rv")
                nc.vector.reciprocal_approx_fast(rv_t[:, :nb], veps_t[:, :nb])
                nc.scalar.activation(
                    negmu_t[:, :nb], ps_s[0:65, :nb], AF.Identity,
                    scale=-1.0 / HIDDEN,
                )
                rsig_t = rv_pool.tile([1, NB], mmdt, tag="rsig")
                nc.scalar.activation(rsig_t[:, :nb], rv_t[:, :nb], AF.Sqrt)

            pending.append(functools.partial(
                emit_tail, t, c0, nb, ph_list, negmu_t, sv_t, rsig_t
            ))

        for pf in pending:
            pf()

    nc.compile()
    return nc


def _tf32(x):
    """Round fp32 to TF32 (10-bit mantissa, round-to-nearest-even)."""
    u = np.ascontiguousarray(x, dtype=np.float32).view(np.uint32).copy()
    lsb = (u >> np.uint32(13)) & np.uint32(1)
    u += np.uint32(0x0FFF) + lsb
    u &= np.uint32(0xFFFFE000)
    return u.view(np.float32)


def _tile_cols(a, kt):
    """[kt*P, C] -> [P, kt*C] with col index = k*C + c (the SBUF layout)."""
    kp, C = a.shape
    assert kp == kt * P
    return np.ascontiguousarray(
        a.reshape(kt, P, C).transpose(1, 0, 2).reshape(P, kt * C)
    )


def _tile_cols_mmajor(a, kt):
    """[kt*P, mt*P] -> [P, mt*kt*P] with col index = m*(kt*P) + k*P + pp,
    so a contiguous column range covers a run of m-chunks for ALL k."""
    kp, C = a.shape
    assert kp == kt * P and C % P == 0
    mt = C // P
    return np.ascontiguousarray(
        a.reshape(kt, P, mt, P).transpose(1, 2, 0, 3).reshape(P, mt * kt * P)
    )


def prep_inputs(node_latent, w1, b1, w2, b2, ln_gamma, ln_beta, head_w, head_b,
                caps, idx_by_type, mm_bf16=True):
    """Build the 8 per-core input maps (everything pre-tiled to SBUF layout)."""
    if mm_bf16:
        import ml_dtypes

        cast = lambda a: np.asarray(a, dtype=np.float32).astype(ml_dtypes.bfloat16)
    else:
        cast = _tf32
    whp = np.asarray(ln_gamma)[:, None] * np.asarray(head_w)  # [T, H, OUT]
    # mc-major per type: col = ((t*MO + mc)*KH + k)*P + pp
    whpp = np.concatenate(
        [_tile_cols_mmajor(cast(whp[t]), KH) for t in range(TYPES)], axis=1
    )  # [P, T*MO*KH*P]
    c1 = cast(np.asarray(ln_beta @ head_w + head_b)).reshape(1, TYPES * OUT)
    c2 = cast(np.asarray(ln_gamma @ head_w)).reshape(1, TYPES * OUT)
    w1p = _tile_cols_mmajor(cast(w1), KL)  # [P, MH*KL*P], m-major
    w2p = _tile_cols_mmajor(cast(w2), KH)  # [P, MH*KH*P], m-major
    b1r = np.ascontiguousarray(np.asarray(b1).reshape(MH, P).T).astype(np.float32)
    b2r = np.ascontiguousarray(np.asarray(b2).reshape(MH, P).T).astype(np.float32)
    R = sum(caps)
    blocks = _blocks_from_caps(caps)
    node_latent = np.asarray(node_latent, dtype=np.float32)
    in_maps = []
    for c in range(N_CORES):
        xc = np.zeros((R, LATENT), np.float32)
        off = 0
        for tt in range(TYPES):
            idx = idx_by_type[tt][c]
            xc[off : off + len(idx)] = node_latent[idx]
            off += caps[tt]
        xcb = cast(xc)
        xtp = np.empty((P, KL * R), dtype=xcb.dtype)
        for (_t, c0, nb) in blocks:
            xtp[:, KL * c0 : KL * (c0 + nb)] = (
                xcb[c0 : c0 + nb, :].reshape(nb, KL, P).transpose(2, 1, 0)
                .reshape(P, KL * nb)
            )
        in_maps.append(
            {
                "xtp": xtp,
                "w1p": w1p,
                "w2p": w2p,
                "whpp": whpp,
                "b1r": b1r,
                "b2r": b2r,
                "c1r": c1,
                "c2r": c2,
            }
        )
    return in_maps


def unpack_outputs(results, caps, idx_by_type, n_rows):
    out = np.empty((n_rows, OUT), np.float32)
    for c in range(N_CORES):
        oc = results[c]["out"]  # [OUT, R]
        off = 0
        for tt in range(TYPES):
            idx = idx_by_type[tt][c]
            out[idx] = oc[:, off : off + len(idx)].T
            off += caps[tt]
    return out


def kernel(node_latent, node_types, w1, b1, w2, b2, ln_gamma, ln_beta, head_w, head_b):
    from concourse.bass_utils import run_bass_kernel_spmd

    node_latent = np.asarray(node_latent, dtype=np.float32)
    node_types = np.asarray(node_types)
    blocks, R, caps, idx_by_type = plan(node_types)
    use_c1 = bool(np.any(np.asarray(ln_beta @ head_w + head_b)))
    use_b1 = bool(np.any(np.asarray(b1)))
    use_b2 = bool(np.any(np.asarray(b2)))
    nc = build_program(
        blocks, R, use_c1=use_c1, use_b1=use_b1, use_b2=use_b2, mm_bf16=MM_BF16
    )
    in_maps = prep_inputs(
        node_latent, w1, b1, w2, b2, ln_gamma, ln_beta, head_w, head_b,
        caps, idx_by_type, mm_bf16=MM_BF16,
    )
    res = run_bass_kernel_spmd(nc, in_maps, core_ids=list(range(N_CORES)))
    return unpack_outputs(res.results, caps, idx_by_type, node_latent.shape[0])
